# revision 1
# baseline (speedup 1.0000x reference)
"""HGNN layer kernel for 8 Trainium2 NeuronCores.

Strategy: shard by destination node. Host cuts the node range into contiguous
variable-size chunks (<=128 nodes, per-type/slot/bank edge caps), assigns an
equal number of chunks to each core (uniform SPMD program). Per chunk, each
edge-type/slot stream is gathered from HBM via dma_gather (4 high-bit banks so
indices fit int16), then a one-hot selection matrix R (built on DVE from dst
positions) turns gather+matmul+segment-sum into:
    H_s   = G_s.T @ R        (PE, accumulated over the slot's tiles in PSUM)
    agg_t = sum_s H_s.T @ W_s  (PE)
    out   = sum_t r_t * agg_t + x@WC.T + bC   (DVE scalar_tensor_tensor)
Normalization r_t = 1/count is host-derived index metadata (like the CSR sort).
No collectives needed: each core owns its chunks' outputs.
"""
import sys, os
sys.path.insert(0, "/opt/trn_rl_repo")
import numpy as np
STAGE = int(os.environ.get("STAGE", "9"))  # 1=gathers only, 2=+R, 3=+H, 4=+agg, 9=all

P = 128
D = 128
NCORES = 8
BANK = 32768
CAPS_T = (2, 2, 2, 1)          # tiles per bank segment (bank3 is the 1696-row tail)
CAPS_SELF = (1, 1, 1, 1)
SLOTS = ((0, 0), (1, 0), (1, 1), (2, 0), (2, 1), (2, 2), (3, 0))  # (type, slot); 3 = self
NSLOT = len(SLOTS)              # 6 edge slots + self
SLOT_CAPS = [CAPS_T] * 6 + [CAPS_SELF]
SLOT_TILES = [sum(c) for c in SLOT_CAPS]
TILES_CHUNK = sum(SLOT_TILES)   # 46
TILE_OFF = np.cumsum([0] + SLOT_TILES).tolist()
G_CH = 2                        # chunks per pipeline group


def _plan_core(node_lo, node_hi, dst_t, srcslot_t, counts_t, caps):
    """Cut [node_lo, node_hi) into chunks and build per-chunk streams."""
    nodes = np.arange(node_lo, node_hi)
    # per-node per-(t,s,b) edge counts for the cutting pass
    percnt = np.zeros((node_hi - node_lo, 6, 4), np.int32)
    for si, (t, s) in enumerate(SLOTS[:6]):
        dst = dst_t[t]
        sel = (dst >= node_lo) & (dst < node_hi)
        b = (srcslot_t[t][s][sel] >> 15).astype(np.int64)
        np.add.at(percnt, (dst[sel] - node_lo, si, np.minimum(b, 3)), 1)
    chunks = []
    i, n = 0, node_hi - node_lo
    segcap = np.array(caps, np.int32) * P
    while i < n:
        acc = np.zeros((6, 4), np.int32)
        j = i
        while j < n and j - i < P:
            nxt = acc + percnt[j]
            if (nxt > segcap[None, :]).any():
                break
            acc = nxt
            j += 1
        if j == i:  # single node exceeding a cap: shouldn't happen at this scale
            j = i + 1
        chunks.append((node_lo + i, node_lo + j))
        i = j
    return chunks


def _build_streams(chunks, nch, dst_t, srcslot_t, counts_t, bank_sizes, sorted_t=None):
    """Per-core stream arrays for the uniform program."""
    ntyp = len(dst_t)
    # index streams per bank (G order: group-major, bank-major inside group)
    ngroups = nch // G_CH
    # within bank b's region (per group): per chunk, slots in order, each cap[si][b]*P
    per_chunk_bank = [sum(SLOT_CAPS[si][b] for si in range(NSLOT)) * P for b in range(4)]
    bank_base = [[sum(SLOT_CAPS[sj][b] for sj in range(si)) * P for si in range(NSLOT)]
                 for b in range(4)]
    bank_region = [G_CH * per_chunk_bank[b] for b in range(4)]
    idx_streams = [np.zeros((ngroups, bank_region[b]), np.int16) for b in range(4)]
    # dst stream (R order: chunk-major; per chunk: slots, then bank segs in order)
    dst_stream = np.full((nch, TILES_CHUNK * P), 999.0, np.float32)
    r_arr = np.zeros((nch, ntyp, P), np.float32)
    selfbase = np.zeros(nch, np.int32)
    meta = []
    for ci in range(nch):
        if ci < len(chunks):
            lo, hi = chunks[ci]
        else:
            lo, hi = 0, 0  # empty pad chunk; selfbase points at pad rows
        meta.append((lo, hi))
        selfbase[ci] = lo if hi > lo else 0
        g, cig = ci // G_CH, ci % G_CH
        for si, (t, s) in enumerate(SLOTS):
            if t < 3:
                sdst, ssrc = sorted_t[t]
                a = np.searchsorted(sdst, lo)
                z = np.searchsorted(sdst, hi)
                e_dst = sdst[a:z] - lo
                e_src = ssrc[s][a:z]
            else:  # self slot: node -> its own position
                e_src = np.arange(lo, hi, dtype=np.int64)
                e_dst = np.arange(hi - lo, dtype=np.int64)
            order = np.argsort(e_src >> 15, kind="stable")
            e_dst, e_src = e_dst[order], e_src[order]
            bank = (e_src >> 15).astype(np.int64)
            dcol0 = TILE_OFF[si] * P
            seg_off = 0
            for b in range(4):
                m = bank == b
                sb = e_src[m] - b * BANK
                db = e_dst[m]
                nb = sb.shape[0]
                caps = SLOT_CAPS[si]
                assert nb <= caps[b] * P, (si, b, nb)
                base = bank_base[b][si] + cig * per_chunk_bank[b]
                idx_streams[b][g, base:base + nb] = sb.astype(np.int16)
                # pads keep 0 (gather bank row 0, dst stays 999)
                dst_stream[ci, dcol0 + seg_off: dcol0 + seg_off + nb] = db.astype(np.float32)
                seg_off += caps[b] * P
        for t in range(ntyp):
            npos = hi - lo
            if npos > 0:
                c = counts_t[t][lo:hi].astype(np.float32)
                r = np.where(c > 0, 1.0 / np.maximum(c, 1.0), 0.0)
                r_arr[ci, t, :npos] = r
    return idx_streams, dst_stream, r_arr, selfbase, meta


def _wrap16(idx_flat):
    """dma_gather index layout: j -> [j%16, j//16], replicated across 8 groups."""
    n = idx_flat.shape[0]
    w = np.zeros((P, n // 16), np.int16)
    j = np.arange(n)
    w[j % 16, j // 16] = idx_flat
    for g in range(1, 8):
        w[g * 16:(g + 1) * 16] = w[0:16]
    return w


def _run(x, dst_t, srcslot_t, W_slots, WCt, bC, n_nodes, core_ids=None, sim=False):
    from concourse import bass, bacc, mybir, tile
    from concourse.bass_utils import run_bass_kernel_spmd

    ntyp = len(dst_t)
    counts_t = [np.bincount(dst_t[t], minlength=n_nodes) for t in range(ntyp)]
    bank_sizes = [min(BANK, max(0, n_nodes - b * BANK)) for b in range(4)]
    nb_banks = sum(1 for s in bank_sizes if s > 0)

    # ---- per-core planning (uniform structure across cores) ----
    per_core = (n_nodes + NCORES - 1) // NCORES
    plans = []
    for c in range(NCORES):
        lo, hi = c * per_core, min((c + 1) * per_core, n_nodes)
        plans.append(_plan_core(lo, hi, dst_t, srcslot_t, counts_t, CAPS_T))
    nch = max(len(p) for p in plans)
    nch += (-nch) % G_CH
    ngroups = nch // G_CH

    sorted_t = []
    for t in range(ntyp):
        o = np.argsort(dst_t[t], kind="stable")
        sorted_t.append((dst_t[t][o], [srcslot_t[t][s][o] for s in range(t + 1)]))
    streams = [_build_streams(plans[c], nch, dst_t, srcslot_t, counts_t, bank_sizes,
                              sorted_t) for c in range(NCORES)]

    per_chunk_bank = [sum(SLOT_CAPS[si][b] for si in range(NSLOT)) * P for b in range(4)]
    bank_base = [[sum(SLOT_CAPS[sj][b] for sj in range(si)) * P for si in range(NSLOT)]
                 for b in range(4)]
    bank_region = [G_CH * per_chunk_bank[b] for b in range(4)]
    bank_tiles = [r // P for r in bank_region]

    x_pad = np.vstack([x, np.zeros((P, D), np.float32)])
    iota = np.tile(np.arange(P, dtype=np.float32), (P, 1))
    ones_row = np.ones((1, P), np.float32)

    # ---- build program ----
    nc = bacc.Bacc("TRN2", target_bir_lowering=False, debug=False)
    dt = mybir.dt
    x_d = nc.declare_dram_parameter("x", [n_nodes + P, D], dt.float32, isOutput=False)
    idx_d = [nc.declare_dram_parameter(f"idx{b}", [ngroups, P, bank_region[b] // 16],
                                       dt.int16, isOutput=False) for b in range(nb_banks)]
    dst_d = nc.declare_dram_parameter("dst", [nch, P, TILES_CHUNK], dt.float32, isOutput=False)
    r_d = nc.declare_dram_parameter("r", [nch, P, ntyp], dt.float32, isOutput=False)
    w_d = nc.declare_dram_parameter("wslots", [NSLOT, P, D], dt.float32, isOutput=False)
    bc_d = nc.declare_dram_parameter("bc", [1, D], dt.float32, isOutput=False)
    io_d = nc.declare_dram_parameter("iota", [P, P], dt.float32, isOutput=False)
    on_d = nc.declare_dram_parameter("ones", [1, P], dt.float32, isOutput=False)
    out_d = nc.declare_dram_parameter("out", [nch * P, D], dt.float32, isOutput=True)

    # self gathers use indirect dma (consecutive rows per chunk): base row per chunk
    AF = mybir.ActivationFunctionType
    AL = mybir.AluOpType

    with tile.TileContext(nc) as tc:
        with (
            tc.tile_pool(name="const", bufs=1) as cpool,
            tc.tile_pool(name="sbuf", bufs=2) as sb,
            tc.tile_pool(name="psum", bufs=2, space="PSUM") as ps,
        ):
            w_t = cpool.tile([P, NSLOT, D], dt.float32)
            nc.sync.dma_start(out=w_t[:], in_=w_d[:].rearrange("w p d -> p w d"))
            io_t = cpool.tile([P, P], dt.float32)
            nc.sync.dma_start(out=io_t[:], in_=io_d[:])
            on_t = cpool.tile([1, P], dt.float32)
            nc.sync.dma_start(out=on_t[:], in_=on_d[:])
            bc_t = cpool.tile([1, P], dt.float32)
            nc.sync.dma_start(out=bc_t[:], in_=bc_d[:])

            for g in range(ngroups):
                gtiles = []
                for b in range(nb_banks):
                    gt = sb.tile([P, bank_tiles[b], D], dt.float32, tag=f"g{b}")
                    it = sb.tile([P, bank_region[b] // 16], dt.int16, tag=f"i{b}")
                    nc.sync.dma_start(out=it[:], in_=idx_d[b][g])
                    if STAGE < 1 or bank_sizes[b] == 0:
                        nc.gpsimd.memset(gt[:], 0.0)
                        gtiles.append(gt)
                        continue
                    GMAX = 1024
                    for off in range(0, bank_region[b], GMAX):
                        n = min(GMAX, bank_region[b] - off)
                        nc.gpsimd.dma_gather(
                            out_ap=gt[:, off // P:(off + n) // P, :],
                            in_ap=x_d[b * BANK: b * BANK + bank_sizes[b], :],
                            idxs_ap=it[:, off // 16:(off + n) // 16],
                            num_idxs=n, num_idxs_reg=n, elem_size=D)
                    gtiles.append(gt)
                dst_tl = sb.tile([P, G_CH, TILES_CHUNK], dt.float32, tag="dst")
                nc.sync.dma_start(out=dst_tl[:], in_=dst_d[:].rearrange(
                    "(g c) p k -> g p c k", c=G_CH)[g])
                r_tl = sb.tile([P, G_CH, ntyp], dt.float32, tag="r")
                nc.sync.dma_start(out=r_tl[:], in_=r_d[:].rearrange(
                    "(g c) p k -> g p c k", c=G_CH)[g])
                out_tl = sb.tile([P, G_CH, D], dt.float32, tag="out")

                for cig in range(G_CH):
                    if STAGE < 2:
                        nc.vector.tensor_copy(out=out_tl[:, cig, :], in_=io_t[:])
                        continue
                    ci = g * G_CH + cig
                    # R build: one DVE op per type over its 14 tiles (2 slots x 7) max;
                    # actually per (t): arity*TILES_SLOT tiles
                    rt_tiles = {}
                    for si in range(NSLOT):
                        nt = SLOT_TILES[si]
                        rt = sb.tile([P, nt, P], dt.float32, tag=f"R{si}")
                        nc.vector.tensor_tensor(
                            out=rt[:],
                            in0=dst_tl[:, cig, TILE_OFF[si]:TILE_OFF[si] + nt, None]
                                .to_broadcast([P, nt, P]),
                            in1=io_t[:, None, :].to_broadcast([P, nt, P]),
                            op=AL.is_equal)
                        rt_tiles[si] = rt
                    if STAGE < 3:
                        pass
                    # H accumulation
                    h_ps_a = ps.tile([P, 4 * P], dt.float32, space="PSUM", tag="ha")
                    h_ps_b = ps.tile([P, 3 * P], dt.float32, space="PSUM", tag="hb")
                    hmap = {}
                    for si in range(NSLOT):
                        if si < 4:
                            hmap[si] = h_ps_a[:, si * P:(si + 1) * P]
                        else:
                            hmap[si] = h_ps_b[:, (si - 4) * P:(si - 3) * P]
                    if STAGE < 3:
                        nc.vector.tensor_copy(out=out_tl[:, cig, :], in_=rt_tiles[0][:, 0, :])
                        continue
                    # one accumulation group per PSUM bank (start zeroes 2KB bank)
                    mm_a = []  # (out_slice, lhsT, rhs) for bank a (slots 0-3)
                    mm_b = []  # bank b (slots 4,5,6)
                    for si in range(NSLOT):
                        k = 0
                        for b in range(nb_banks):
                            base_t = (bank_base[b][si] + cig * per_chunk_bank[b]) // P
                            for tb in range(SLOT_CAPS[si][b]):
                                trip = (hmap[si], gtiles[b][:, base_t + tb, :],
                                        rt_tiles[si][:, k, :])
                                (mm_a if si < 4 else mm_b).append(trip)
                                k += 1
                    for mms in (mm_a, mm_b):
                        for i, (o, l, rr_) in enumerate(mms):
                            nc.tensor.matmul(out=o, lhsT=l, rhs=rr_,
                                             start=(i == 0), stop=(i == len(mms) - 1))
                    if STAGE < 4:
                        nc.scalar.activation(out=out_tl[:, cig, :], in_=h_ps_a[:, 0:P], func=AF.Copy)
                        continue
                    h_sb_a = sb.tile([P, 4 * P], dt.float32, tag="hsa")
                    nc.scalar.activation(out=h_sb_a[:], in_=h_ps_a[:], func=AF.Copy)
                    h_sb_b = sb.tile([P, 3 * P], dt.float32, tag="hsb")
                    nc.scalar.activation(out=h_sb_b[:], in_=h_ps_b[:], func=AF.Copy)
                    hs = {}
                    for si in range(NSLOT):
                        if si < 4:
                            hs[si] = h_sb_a[:, si * P:(si + 1) * P]
                        else:
                            hs[si] = h_sb_b[:, (si - 4) * P:(si - 3) * P]
                    # agg psum: [t0, t1, t2, self]
                    agg = ps.tile([P, 4 * P], dt.float32, space="PSUM", tag="agg")
                    mm_g = [(agg[:, 3 * P:4 * P], on_t[:], bc_t[:]),
                            (agg[:, 3 * P:4 * P], hs[NSLOT - 1], w_t[:, NSLOT - 1, :])]
                    slot_of_type = {0: [0], 1: [1, 2], 2: [3, 4, 5]}
                    for t in range(ntyp):
                        for si in slot_of_type[t]:
                            mm_g.append((agg[:, t * P:(t + 1) * P], hs[si], w_t[:, si, :]))
                    for i, (o, l, rr_) in enumerate(mm_g):
                        nc.tensor.matmul(out=o, lhsT=l, rhs=rr_,
                                         start=(i == 0), stop=(i == len(mm_g) - 1))
                    # combine: out = self + sum_t r_t * agg_t  (one PSUM input per op)
                    nc.scalar.activation(out=out_tl[:, cig, :], in_=agg[:, 3 * P:4 * P],
                                         func=AF.Copy)
                    for t in range(0, ntyp):
                        nc.vector.scalar_tensor_tensor(
                            out=out_tl[:, cig, :], in0=agg[:, t * P:(t + 1) * P],
                            scalar=r_tl[:, cig, t:t + 1], in1=out_tl[:, cig, :],
                            op0=AL.mult, op1=AL.add)
                nc.sync.dma_start(
                    out=out_d[:].rearrange("(g c p) d -> g p c d", c=G_CH, p=P)[g],
                    in_=out_tl[:])
    nc.finalize()

    in_maps = []
    for c in range(NCORES):
        idx_streams, dst_stream, r_arr, selfbase, meta = streams[c]
        m = dict(x=x_pad, dst=dst_stream.reshape(nch, TILES_CHUNK, P)
                 .transpose(0, 2, 1).copy(),
                 r=r_arr.transpose(0, 2, 1).copy(),
                 wslots=W_slots, bc=bC.reshape(1, D),
                 iota=iota, ones=ones_row)
        for b in range(nb_banks):
            m[f"idx{b}"] = np.stack([_wrap16(idx_streams[b][g]) for g in range(ngroups)])
        in_maps.append(m)

    if sim:
        from concourse import bass_interp
        s = bass_interp.MultiCoreSim(nc, NCORES)
        for c in range(NCORES):
            for k, v in in_maps[c].items():
                s.cores[c].tensor(k)[:] = v
        s.simulate()
        results = [{"out": np.asarray(s.cores[c].tensor("out")).copy()}
                   for c in range(NCORES)]
        rr = type("R", (), {})(); rr.results = results; rr.exec_time_ns = None
    else:
        import time as _time
        rr = run_bass_kernel_spmd(nc, in_maps, core_ids=list(range(NCORES)))
        if os.environ.get("KBENCH", "0") == "1":
            t0 = _time.time()
            rr = run_bass_kernel_spmd(nc, in_maps, core_ids=list(range(NCORES)))
            t1 = _time.time()
            print(f"warm call wall: {(t1-t0)*1e3:.1f} ms")
            t0 = _time.time()
            rr = run_bass_kernel_spmd(nc, in_maps, core_ids=list(range(NCORES)))
            t1 = _time.time()
            print(f"warm call 2 wall: {(t1-t0)*1e3:.1f} ms")
            print(f"HW exec time: {int((t1-t0)*1e9)} ns")

    out_full = np.zeros((n_nodes, D), np.float32)
    for c in range(NCORES):
        _, _, _, _, meta = streams[c]
        o = rr.results[c]["out"].reshape(nch, P, D)
        for ci, (lo, hi) in enumerate(meta):
            if hi > lo:
                out_full[lo:hi] = o[ci, :hi - lo]
    return out_full, rr


def kernel(x, src0, dst0, src1, dst1, src2, dst2, WA0, WA1, WA2, WC, bC):
    x = np.asarray(x, np.float32)
    n_nodes = x.shape[0]
    dst_t = [np.asarray(d, np.int64) for d in (dst0, dst1, dst2)]
    srcs = [np.asarray(s, np.int64) for s in (src0, src1, src2)]
    srcslot_t = [[srcs[t].reshape(-1, t + 1)[:, s] for s in range(t + 1)]
                 for t in range(3)]
    W_slots = np.stack([
        np.asarray(WA0, np.float32)[0:P],
        np.asarray(WA1, np.float32)[0:P], np.asarray(WA1, np.float32)[P:2 * P],
        np.asarray(WA2, np.float32)[0:P], np.asarray(WA2, np.float32)[P:2 * P],
        np.asarray(WA2, np.float32)[2 * P:3 * P],
        np.asarray(WC, np.float32).T.copy(),
    ])
    out, _ = _run(x, dst_t, srcslot_t, W_slots, None, np.asarray(bC, np.float32),
                  n_nodes)
    return out



# revision 2
# speedup vs baseline: 1.6033x; 1.6033x over previous
"""HGNN layer kernel for 8 Trainium2 NeuronCores.

Strategy: shard by destination node. Host cuts the node range into contiguous
variable-size chunks (<=128 nodes, per-type/slot/bank edge caps), assigns an
equal number of chunks to each core (uniform SPMD program). x is shipped
SHARDED (1/8th per core, bf16) and AllGathered on-device into a DRAM scratch
to keep the host->device wire traffic minimal (the axon PJRT tunnel is the
wall-clock bottleneck, ~35 MB/s). Per chunk, each edge-type/slot stream is
gathered from the scratch via dma_gather (4 high-bit banks so indices fit
int16; index tables ship 16-partition-packed and are replicated to 128
partitions on device), then a one-hot selection matrix R (built on DVE from
u8 dst positions) turns gather+matmul+segment-sum into:
    H_s   = G_s.T @ R        (PE bf16, accumulated over the slot's tiles in PSUM)
    agg_t = sum_s H_s.T @ W_s  (PE f32)
    out   = sum_t r_t * agg_t + x@WC.T + bC   (DVE scalar_tensor_tensor)
Normalization r_t = 1/count is host-derived index metadata (like the CSR sort).
No output collectives needed: each core owns its chunks' outputs (bf16 on the
wire, f32 on host).
"""
import sys, os
sys.path.insert(0, "/opt/trn_rl_repo")
import numpy as np
import ml_dtypes
STAGE = int(os.environ.get("STAGE", "9"))  # 1=gathers only, 2=+R, 3=+H, 4=+agg, 9=all

P = 128
D = 128
NCORES = 8
BANK = 32768
CAPS_T = (2, 2, 2, 1)          # tiles per bank segment (bank3 is the 1696-row tail)
CAPS_SELF = (1, 1, 1, 1)
SLOTS = ((0, 0), (1, 0), (1, 1), (2, 0), (2, 1), (2, 2), (3, 0))  # (type, slot); 3 = self
NSLOT = len(SLOTS)              # 6 edge slots + self
SLOT_CAPS = [CAPS_T] * 6 + [CAPS_SELF]
SLOT_TILES = [sum(c) for c in SLOT_CAPS]
TILES_CHUNK = sum(SLOT_TILES)   # 46
TILE_OFF = np.cumsum([0] + SLOT_TILES).tolist()
G_CH = 2                        # chunks per pipeline group
DST_PAD = 255                   # u8 pad marker (>= P, matches no iota value)


def _plan_core(node_lo, node_hi, dst_t, srcslot_t, counts_t, caps):
    """Cut [node_lo, node_hi) into chunks and build per-chunk streams."""
    nodes = np.arange(node_lo, node_hi)
    # per-node per-(t,s,b) edge counts for the cutting pass
    percnt = np.zeros((node_hi - node_lo, 6, 4), np.int32)
    for si, (t, s) in enumerate(SLOTS[:6]):
        dst = dst_t[t]
        sel = (dst >= node_lo) & (dst < node_hi)
        b = (srcslot_t[t][s][sel] >> 15).astype(np.int64)
        np.add.at(percnt, (dst[sel] - node_lo, si, np.minimum(b, 3)), 1)
    chunks = []
    i, n = 0, node_hi - node_lo
    segcap = np.array(caps, np.int32) * P
    while i < n:
        acc = np.zeros((6, 4), np.int32)
        j = i
        while j < n and j - i < P:
            nxt = acc + percnt[j]
            if (nxt > segcap[None, :]).any():
                break
            acc = nxt
            j += 1
        if j == i:  # single node exceeding a cap: shouldn't happen at this scale
            j = i + 1
        chunks.append((node_lo + i, node_lo + j))
        i = j
    return chunks


def _build_streams(chunks, nch, dst_t, srcslot_t, counts_t, bank_sizes, sorted_t=None):
    """Per-core stream arrays for the uniform program."""
    ntyp = len(dst_t)
    # index streams per bank (G order: group-major, bank-major inside group)
    ngroups = nch // G_CH
    # within bank b's region (per group): per chunk, slots in order, each cap[si][b]*P
    per_chunk_bank = [sum(SLOT_CAPS[si][b] for si in range(NSLOT)) * P for b in range(4)]
    bank_base = [[sum(SLOT_CAPS[sj][b] for sj in range(si)) * P for si in range(NSLOT)]
                 for b in range(4)]
    bank_region = [G_CH * per_chunk_bank[b] for b in range(4)]
    idx_streams = [np.zeros((ngroups, bank_region[b]), np.int16) for b in range(4)]
    # dst stream (R order: chunk-major; per chunk: slots, then bank segs in order)
    dst_stream = np.full((nch, TILES_CHUNK * P), DST_PAD, np.uint8)
    r_arr = np.zeros((nch, ntyp, P), np.float32)
    selfbase = np.zeros(nch, np.int32)
    meta = []
    for ci in range(nch):
        if ci < len(chunks):
            lo, hi = chunks[ci]
        else:
            lo, hi = 0, 0  # empty pad chunk; selfbase points at pad rows
        meta.append((lo, hi))
        selfbase[ci] = lo if hi > lo else 0
        g, cig = ci // G_CH, ci % G_CH
        for si, (t, s) in enumerate(SLOTS):
            if t < 3:
                sdst, ssrc = sorted_t[t]
                a = np.searchsorted(sdst, lo)
                z = np.searchsorted(sdst, hi)
                e_dst = sdst[a:z] - lo
                e_src = ssrc[s][a:z]
            else:  # self slot: node -> its own position
                e_src = np.arange(lo, hi, dtype=np.int64)
                e_dst = np.arange(hi - lo, dtype=np.int64)
            order = np.argsort(e_src >> 15, kind="stable")
            e_dst, e_src = e_dst[order], e_src[order]
            bank = (e_src >> 15).astype(np.int64)
            dcol0 = TILE_OFF[si] * P
            seg_off = 0
            for b in range(4):
                m = bank == b
                sb = e_src[m] - b * BANK
                db = e_dst[m]
                nb = sb.shape[0]
                caps = SLOT_CAPS[si]
                assert nb <= caps[b] * P, (si, b, nb)
                base = bank_base[b][si] + cig * per_chunk_bank[b]
                idx_streams[b][g, base:base + nb] = sb.astype(np.int16)
                # pads keep 0 (gather bank row 0, dst stays DST_PAD)
                dst_stream[ci, dcol0 + seg_off: dcol0 + seg_off + nb] = db.astype(np.uint8)
                seg_off += caps[b] * P
        for t in range(ntyp):
            npos = hi - lo
            if npos > 0:
                c = counts_t[t][lo:hi].astype(np.float32)
                r = np.where(c > 0, 1.0 / np.maximum(c, 1.0), 0.0)
                r_arr[ci, t, :npos] = r
    return idx_streams, dst_stream, r_arr, selfbase, meta


def _wrap16(idx_flat):
    """dma_gather index layout: j -> [j%16, j//16]; device replicates to 128 parts."""
    n = idx_flat.shape[0]
    w = np.zeros((16, n // 16), np.int16)
    j = np.arange(n)
    w[j % 16, j // 16] = idx_flat
    return w


def _run(x, dst_t, srcslot_t, W_slots, WCt, bC, n_nodes, core_ids=None, sim=False):
    from concourse import bass, bacc, mybir, tile
    from concourse.bass_utils import run_bass_kernel_spmd

    ntyp = len(dst_t)
    counts_t = [np.bincount(dst_t[t], minlength=n_nodes) for t in range(ntyp)]
    bank_sizes = [min(BANK, max(0, n_nodes - b * BANK)) for b in range(4)]
    nb_banks = sum(1 for s in bank_sizes if s > 0)
    n_pad = n_nodes + P
    assert n_pad % NCORES == 0
    shard = n_pad // NCORES

    # ---- per-core planning (uniform structure across cores) ----
    per_core = (n_nodes + NCORES - 1) // NCORES
    plans = []
    for c in range(NCORES):
        lo, hi = c * per_core, min((c + 1) * per_core, n_nodes)
        plans.append(_plan_core(lo, hi, dst_t, srcslot_t, counts_t, CAPS_T))
    nch = max(len(p) for p in plans)
    nch += (-nch) % G_CH
    ngroups = nch // G_CH

    sorted_t = []
    for t in range(ntyp):
        o = np.argsort(dst_t[t], kind="stable")
        sorted_t.append((dst_t[t][o], [srcslot_t[t][s][o] for s in range(t + 1)]))
    streams = [_build_streams(plans[c], nch, dst_t, srcslot_t, counts_t, bank_sizes,
                              sorted_t) for c in range(NCORES)]

    per_chunk_bank = [sum(SLOT_CAPS[si][b] for si in range(NSLOT)) * P for b in range(4)]
    bank_base = [[sum(SLOT_CAPS[sj][b] for sj in range(si)) * P for si in range(NSLOT)]
                 for b in range(4)]
    bank_region = [G_CH * per_chunk_bank[b] for b in range(4)]
    bank_tiles = [r // P for r in bank_region]

    x_pad = np.vstack([x, np.zeros((P, D), np.float32)])
    x_bf = x_pad.astype(ml_dtypes.bfloat16)
    iota = np.tile(np.arange(P, dtype=np.float32), (P, 1))
    ones_row = np.ones((1, P), np.float32)

    # ---- build program ----
    nc = bacc.Bacc("TRN2", target_bir_lowering=False, debug=False,
                   num_devices=NCORES)
    dt = mybir.dt
    xs_d = nc.declare_dram_parameter("xs", [shard, D], dt.bfloat16, isOutput=False)
    idx_d = [nc.declare_dram_parameter(f"idx{b}", [ngroups, 16, bank_region[b] // 16],
                                       dt.int16, isOutput=False) for b in range(nb_banks)]
    dst_d = nc.declare_dram_parameter("dst", [nch, P, TILES_CHUNK], dt.uint8, isOutput=False)
    r_d = nc.declare_dram_parameter("r", [nch, P, ntyp], dt.float32, isOutput=False)
    w_d = nc.declare_dram_parameter("wslots", [NSLOT, P, D], dt.float32, isOutput=False)
    bc_d = nc.declare_dram_parameter("bc", [1, D], dt.float32, isOutput=False)
    io_d = nc.declare_dram_parameter("iota", [P, P], dt.float32, isOutput=False)
    on_d = nc.declare_dram_parameter("ones", [1, P], dt.float32, isOutput=False)
    out_d = nc.declare_dram_parameter("out", [nch * P, D], dt.bfloat16, isOutput=True)

    AF = mybir.ActivationFunctionType
    AL = mybir.AluOpType

    with tile.TileContext(nc) as tc:
        with (
            tc.tile_pool(name="const", bufs=1) as cpool,
            tc.tile_pool(name="dram", bufs=1, space="DRAM") as dram,
            tc.tile_pool(name="sbuf", bufs=2) as sb,
            tc.tile_pool(name="psum", bufs=2, space="PSUM") as ps,
        ):
            # x: shard -> bounce -> AllGather into full bf16 scratch
            x_in = dram.tile([shard, D], dt.bfloat16)
            x_full = dram.tile([n_pad, D], dt.bfloat16)
            nc.gpsimd.dma_start(out=x_in[:], in_=xs_d[:])
            nc.gpsimd.collective_compute(
                "AllGather", AL.bypass,
                replica_groups=[list(range(NCORES))],
                ins=[x_in[:].opt()], outs=[x_full[:].opt()])

            w_t = cpool.tile([P, NSLOT, D], dt.float32)
            nc.sync.dma_start(out=w_t[:], in_=w_d[:].rearrange("w p d -> p w d"))
            io_t = cpool.tile([P, P], dt.float32)
            nc.sync.dma_start(out=io_t[:], in_=io_d[:])
            on_t = cpool.tile([1, P], dt.float32)
            nc.sync.dma_start(out=on_t[:], in_=on_d[:])
            bc_t = cpool.tile([1, P], dt.float32)
            nc.sync.dma_start(out=bc_t[:], in_=bc_d[:])

            for g in range(ngroups):
                gtiles = []
                for b in range(nb_banks):
                    gt = sb.tile([P, bank_tiles[b], D], dt.bfloat16, tag=f"g{b}")
                    it = sb.tile([P, bank_region[b] // 16], dt.int16, tag=f"i{b}")
                    nc.sync.dma_start(out=it[0:16, :], in_=idx_d[b][g])
                    nc.sync.dma_start(out=it[16:32, :], in_=it[0:16, :])
                    nc.sync.dma_start(out=it[32:64, :], in_=it[0:32, :])
                    nc.sync.dma_start(out=it[64:128, :], in_=it[0:64, :])
                    if STAGE < 1 or bank_sizes[b] == 0:
                        nc.gpsimd.memset(gt[:], 0.0)
                        gtiles.append(gt)
                        continue
                    GMAX = 1024
                    for off in range(0, bank_region[b], GMAX):
                        n = min(GMAX, bank_region[b] - off)
                        nc.gpsimd.dma_gather(
                            out_ap=gt[:, off // P:(off + n) // P, :],
                            in_ap=x_full[b * BANK: b * BANK + bank_sizes[b], :],
                            idxs_ap=it[:, off // 16:(off + n) // 16],
                            num_idxs=n, num_idxs_reg=n, elem_size=D)
                    gtiles.append(gt)
                dst_u8 = sb.tile([P, G_CH, TILES_CHUNK], dt.uint8, tag="dst8")
                nc.sync.dma_start(out=dst_u8[:], in_=dst_d[:].rearrange(
                    "(g c) p k -> g p c k", c=G_CH)[g])
                dst_tl = sb.tile([P, G_CH, TILES_CHUNK], dt.float32, tag="dst")
                nc.vector.tensor_copy(out=dst_tl[:], in_=dst_u8[:])
                r_tl = sb.tile([P, G_CH, ntyp], dt.float32, tag="r")
                nc.sync.dma_start(out=r_tl[:], in_=r_d[:].rearrange(
                    "(g c) p k -> g p c k", c=G_CH)[g])
                out_tl = sb.tile([P, G_CH, D], dt.float32, tag="out")
                outb_tl = sb.tile([P, G_CH, D], dt.bfloat16, tag="outb")

                for cig in range(G_CH):
                    if STAGE < 2:
                        nc.vector.tensor_copy(out=out_tl[:, cig, :], in_=io_t[:])
                        continue
                    ci = g * G_CH + cig
                    # R build: one DVE op per slot over its tiles
                    rt_tiles = {}
                    for si in range(NSLOT):
                        nt = SLOT_TILES[si]
                        rt = sb.tile([P, nt, P], dt.bfloat16, tag=f"R{si}")
                        nc.vector.tensor_tensor(
                            out=rt[:],
                            in0=dst_tl[:, cig, TILE_OFF[si]:TILE_OFF[si] + nt, None]
                                .to_broadcast([P, nt, P]),
                            in1=io_t[:, None, :].to_broadcast([P, nt, P]),
                            op=AL.is_equal)
                        rt_tiles[si] = rt
                    if STAGE < 3:
                        pass
                    # H accumulation
                    h_ps_a = ps.tile([P, 4 * P], dt.float32, space="PSUM", tag="ha")
                    h_ps_b = ps.tile([P, 3 * P], dt.float32, space="PSUM", tag="hb")
                    hmap = {}
                    for si in range(NSLOT):
                        if si < 4:
                            hmap[si] = h_ps_a[:, si * P:(si + 1) * P]
                        else:
                            hmap[si] = h_ps_b[:, (si - 4) * P:(si - 3) * P]
                    if STAGE < 3:
                        nc.vector.tensor_copy(out=out_tl[:, cig, :], in_=rt_tiles[0][:, 0, :])
                        continue
                    # one accumulation group per PSUM bank (start zeroes 2KB bank)
                    mm_a = []  # (out_slice, lhsT, rhs) for bank a (slots 0-3)
                    mm_b = []  # bank b (slots 4,5,6)
                    for si in range(NSLOT):
                        k = 0
                        for b in range(nb_banks):
                            base_t = (bank_base[b][si] + cig * per_chunk_bank[b]) // P
                            for tb in range(SLOT_CAPS[si][b]):
                                trip = (hmap[si], gtiles[b][:, base_t + tb, :],
                                        rt_tiles[si][:, k, :])
                                (mm_a if si < 4 else mm_b).append(trip)
                                k += 1
                    for mms in (mm_a, mm_b):
                        for i, (o, l, rr_) in enumerate(mms):
                            nc.tensor.matmul(out=o, lhsT=l, rhs=rr_,
                                             start=(i == 0), stop=(i == len(mms) - 1))
                    if STAGE < 4:
                        nc.scalar.activation(out=out_tl[:, cig, :], in_=h_ps_a[:, 0:P], func=AF.Copy)
                        continue
                    h_sb_a = sb.tile([P, 4 * P], dt.float32, tag="hsa")
                    nc.scalar.activation(out=h_sb_a[:], in_=h_ps_a[:], func=AF.Copy)
                    h_sb_b = sb.tile([P, 3 * P], dt.float32, tag="hsb")
                    nc.scalar.activation(out=h_sb_b[:], in_=h_ps_b[:], func=AF.Copy)
                    hs = {}
                    for si in range(NSLOT):
                        if si < 4:
                            hs[si] = h_sb_a[:, si * P:(si + 1) * P]
                        else:
                            hs[si] = h_sb_b[:, (si - 4) * P:(si - 3) * P]
                    # agg psum: [t0, t1, t2, self]
                    agg = ps.tile([P, 4 * P], dt.float32, space="PSUM", tag="agg")
                    mm_g = [(agg[:, 3 * P:4 * P], on_t[:], bc_t[:]),
                            (agg[:, 3 * P:4 * P], hs[NSLOT - 1], w_t[:, NSLOT - 1, :])]
                    slot_of_type = {0: [0], 1: [1, 2], 2: [3, 4, 5]}
                    for t in range(ntyp):
                        for si in slot_of_type[t]:
                            mm_g.append((agg[:, t * P:(t + 1) * P], hs[si], w_t[:, si, :]))
                    for i, (o, l, rr_) in enumerate(mm_g):
                        nc.tensor.matmul(out=o, lhsT=l, rhs=rr_,
                                         start=(i == 0), stop=(i == len(mm_g) - 1))
                    # combine: out = self + sum_t r_t * agg_t  (one PSUM input per op)
                    nc.scalar.activation(out=out_tl[:, cig, :], in_=agg[:, 3 * P:4 * P],
                                         func=AF.Copy)
                    for t in range(0, ntyp):
                        nc.vector.scalar_tensor_tensor(
                            out=out_tl[:, cig, :], in0=agg[:, t * P:(t + 1) * P],
                            scalar=r_tl[:, cig, t:t + 1], in1=out_tl[:, cig, :],
                            op0=AL.mult, op1=AL.add)
                nc.vector.tensor_copy(out=outb_tl[:], in_=out_tl[:])
                nc.sync.dma_start(
                    out=out_d[:].rearrange("(g c p) d -> g p c d", c=G_CH, p=P)[g],
                    in_=outb_tl[:])
    nc.finalize()

    in_maps = []
    for c in range(NCORES):
        idx_streams, dst_stream, r_arr, selfbase, meta = streams[c]
        m = dict(xs=np.ascontiguousarray(x_bf[c * shard:(c + 1) * shard]),
                 dst=dst_stream.reshape(nch, TILES_CHUNK, P)
                 .transpose(0, 2, 1).copy(),
                 r=r_arr.transpose(0, 2, 1).copy(),
                 wslots=W_slots, bc=bC.reshape(1, D),
                 iota=iota, ones=ones_row)
        for b in range(nb_banks):
            m[f"idx{b}"] = np.stack([_wrap16(idx_streams[b][g]) for g in range(ngroups)])
        in_maps.append(m)

    if sim:
        from concourse import bass_interp
        s = bass_interp.MultiCoreSim(nc, NCORES)
        for c in range(NCORES):
            for k, v in in_maps[c].items():
                s.cores[c].tensor(k)[:] = v
        s.simulate()
        results = [{"out": np.asarray(s.cores[c].tensor("out")).copy()}
                   for c in range(NCORES)]
        rr = type("R", (), {})(); rr.results = results; rr.exec_time_ns = None
    else:
        import time as _time
        rr = run_bass_kernel_spmd(nc, in_maps, core_ids=list(range(NCORES)))
        if os.environ.get("KBENCH", "0") == "1":
            t0 = _time.time()
            rr = run_bass_kernel_spmd(nc, in_maps, core_ids=list(range(NCORES)))
            t1 = _time.time()
            print(f"warm call wall: {(t1-t0)*1e3:.1f} ms")
            t0 = _time.time()
            rr = run_bass_kernel_spmd(nc, in_maps, core_ids=list(range(NCORES)))
            t1 = _time.time()
            print(f"warm call 2 wall: {(t1-t0)*1e3:.1f} ms")
            print(f"HW exec time: {int((t1-t0)*1e9)} ns")

    out_full = np.zeros((n_nodes, D), np.float32)
    for c in range(NCORES):
        _, _, _, _, meta = streams[c]
        o = np.asarray(rr.results[c]["out"])
        if o.dtype != ml_dtypes.bfloat16:
            o = o.view(ml_dtypes.bfloat16)
        o = o.astype(np.float32).reshape(nch, P, D)
        for ci, (lo, hi) in enumerate(meta):
            if hi > lo:
                out_full[lo:hi] = o[ci, :hi - lo]
    return out_full, rr


def kernel(x, src0, dst0, src1, dst1, src2, dst2, WA0, WA1, WA2, WC, bC):
    x = np.asarray(x, np.float32)
    n_nodes = x.shape[0]
    dst_t = [np.asarray(d, np.int64) for d in (dst0, dst1, dst2)]
    srcs = [np.asarray(s, np.int64) for s in (src0, src1, src2)]
    srcslot_t = [[srcs[t].reshape(-1, t + 1)[:, s] for s in range(t + 1)]
                 for t in range(3)]
    W_slots = np.stack([
        np.asarray(WA0, np.float32)[0:P],
        np.asarray(WA1, np.float32)[0:P], np.asarray(WA1, np.float32)[P:2 * P],
        np.asarray(WA2, np.float32)[0:P], np.asarray(WA2, np.float32)[P:2 * P],
        np.asarray(WA2, np.float32)[2 * P:3 * P],
        np.asarray(WC, np.float32).T.copy(),
    ])
    out, _ = _run(x, dst_t, srcslot_t, W_slots, None, np.asarray(bC, np.float32),
                  n_nodes)
    return out


# revision 3
# speedup vs baseline: 1.6410x; 1.0235x over previous
"""HGNN layer kernel for 8 Trainium2 NeuronCores.

Strategy: shard by destination node. Host cuts the node range into contiguous
variable-size chunks (<=128 nodes, per-type/slot/bank edge caps), assigns an
equal number of chunks to each core (uniform SPMD program). x is shipped
SHARDED (1/8th per core, bf16) and AllGathered on-device into a DRAM scratch
to keep the host->device wire traffic minimal (the axon PJRT tunnel is the
wall-clock bottleneck, ~35 MB/s). Per chunk, each edge-type/slot stream is
gathered from the scratch via dma_gather (4 high-bit banks so indices fit
int16; index tables ship 16-partition-packed and are replicated to 128
partitions on device), then a one-hot selection matrix R (built on DVE from
u8 dst positions) turns gather+matmul+segment-sum into:
    H_s   = G_s.T @ R        (PE bf16, accumulated over the slot's tiles in PSUM)
    agg_t = sum_s H_s.T @ W_s  (PE f32)
    out   = sum_t r_t * agg_t + x@WC.T + bC   (DVE scalar_tensor_tensor)
Normalization r_t = 1/count is host-derived index metadata (like the CSR sort).
No output collectives needed: each core owns its chunks' outputs (bf16 on the
wire, f32 on host).
"""
import sys, os
sys.path.insert(0, "/opt/trn_rl_repo")
import numpy as np
import ml_dtypes
STAGE = int(os.environ.get("STAGE", "9"))  # 1=gathers only, 2=+R, 3=+H, 4=+agg, 9=all

P = 128
D = 128
NCORES = 8
BANK = 32768
CAPS_T = (2, 2, 2, 1)          # tiles per bank segment (bank3 is the 1696-row tail)
CAPS_SELF = (1, 1, 1, 1)
SLOTS = ((0, 0), (1, 0), (1, 1), (2, 0), (2, 1), (2, 2), (3, 0))  # (type, slot); 3 = self
NSLOT = len(SLOTS)              # 6 edge slots + self
SLOT_CAPS = [CAPS_T] * 6 + [CAPS_SELF]
SLOT_TILES = [sum(c) for c in SLOT_CAPS]
TILES_CHUNK = sum(SLOT_TILES)   # 46
TILE_OFF = np.cumsum([0] + SLOT_TILES).tolist()
G_CH = 2                        # chunks per pipeline group
DST_PAD = 255                   # u8 pad marker (>= P, matches no iota value)


def _plan_core(node_lo, node_hi, dst_t, srcslot_t, counts_t, caps):
    """Cut [node_lo, node_hi) into chunks and build per-chunk streams."""
    nodes = np.arange(node_lo, node_hi)
    # per-node per-(t,s,b) edge counts for the cutting pass
    percnt = np.zeros((node_hi - node_lo, 6, 4), np.int32)
    for si, (t, s) in enumerate(SLOTS[:6]):
        dst = dst_t[t]
        sel = (dst >= node_lo) & (dst < node_hi)
        b = (srcslot_t[t][s][sel] >> 15).astype(np.int64)
        np.add.at(percnt, (dst[sel] - node_lo, si, np.minimum(b, 3)), 1)
    chunks = []
    i, n = 0, node_hi - node_lo
    segcap = np.array(caps, np.int32) * P
    while i < n:
        acc = np.zeros((6, 4), np.int32)
        j = i
        while j < n and j - i < P:
            nxt = acc + percnt[j]
            if (nxt > segcap[None, :]).any():
                break
            acc = nxt
            j += 1
        if j == i:  # single node exceeding a cap: shouldn't happen at this scale
            j = i + 1
        chunks.append((node_lo + i, node_lo + j))
        i = j
    return chunks


def _build_streams(chunks, nch, dst_t, srcslot_t, counts_t, bank_sizes, sorted_t=None):
    """Per-core stream arrays for the uniform program."""
    ntyp = len(dst_t)
    # index streams per bank (G order: group-major, bank-major inside group)
    ngroups = nch // G_CH
    # within bank b's region (per group): per chunk, slots in order, each cap[si][b]*P
    per_chunk_bank = [sum(SLOT_CAPS[si][b] for si in range(NSLOT)) * P for b in range(4)]
    bank_base = [[sum(SLOT_CAPS[sj][b] for sj in range(si)) * P for si in range(NSLOT)]
                 for b in range(4)]
    bank_region = [G_CH * per_chunk_bank[b] for b in range(4)]
    idx_streams = [np.zeros((ngroups, bank_region[b]), np.int16) for b in range(4)]
    # dst stream (R order: chunk-major; per chunk: slots, then bank segs in order)
    dst_stream = np.full((nch, TILES_CHUNK * P), DST_PAD, np.uint8)
    r_arr = np.zeros((nch, ntyp, P), np.float32)
    selfbase = np.zeros(nch, np.int32)
    meta = []
    for ci in range(nch):
        if ci < len(chunks):
            lo, hi = chunks[ci]
        else:
            lo, hi = 0, 0  # empty pad chunk; selfbase points at pad rows
        meta.append((lo, hi))
        selfbase[ci] = lo if hi > lo else 0
        g, cig = ci // G_CH, ci % G_CH
        for si, (t, s) in enumerate(SLOTS):
            if t < 3:
                sdst, ssrc = sorted_t[t]
                a = np.searchsorted(sdst, lo)
                z = np.searchsorted(sdst, hi)
                e_dst = sdst[a:z] - lo
                e_src = ssrc[s][a:z]
            else:  # self slot: node -> its own position
                e_src = np.arange(lo, hi, dtype=np.int64)
                e_dst = np.arange(hi - lo, dtype=np.int64)
            order = np.argsort(e_src >> 15, kind="stable")
            e_dst, e_src = e_dst[order], e_src[order]
            bank = (e_src >> 15).astype(np.int64)
            dcol0 = TILE_OFF[si] * P
            seg_off = 0
            for b in range(4):
                m = bank == b
                sb = e_src[m] - b * BANK
                db = e_dst[m]
                nb = sb.shape[0]
                caps = SLOT_CAPS[si]
                assert nb <= caps[b] * P, (si, b, nb)
                base = bank_base[b][si] + cig * per_chunk_bank[b]
                idx_streams[b][g, base:base + nb] = sb.astype(np.int16)
                # pads keep 0 (gather bank row 0, dst stays DST_PAD)
                dst_stream[ci, dcol0 + seg_off: dcol0 + seg_off + nb] = db.astype(np.uint8)
                seg_off += caps[b] * P
        for t in range(ntyp):
            npos = hi - lo
            if npos > 0:
                c = counts_t[t][lo:hi].astype(np.float32)
                r = np.where(c > 0, 1.0 / np.maximum(c, 1.0), 0.0)
                r_arr[ci, t, :npos] = r
    return idx_streams, dst_stream, r_arr, selfbase, meta


def _wrap16(idx_flat):
    """dma_gather index layout: j -> [j%16, j//16]; device replicates to 128 parts."""
    n = idx_flat.shape[0]
    w = np.zeros((16, n // 16), np.int16)
    j = np.arange(n)
    w[j % 16, j // 16] = idx_flat
    return w


def _run(x, dst_t, srcslot_t, W_slots, WCt, bC, n_nodes, core_ids=None, sim=False):
    from concourse import bass, bacc, mybir, tile
    from concourse.bass_utils import run_bass_kernel_spmd

    ntyp = len(dst_t)
    counts_t = [np.bincount(dst_t[t], minlength=n_nodes) for t in range(ntyp)]
    bank_sizes = [min(BANK, max(0, n_nodes - b * BANK)) for b in range(4)]
    nb_banks = sum(1 for s in bank_sizes if s > 0)
    n_pad = n_nodes + P
    assert n_pad % NCORES == 0
    shard = n_pad // NCORES

    # ---- per-core planning (uniform structure across cores) ----
    per_core = (n_nodes + NCORES - 1) // NCORES
    plans = []
    for c in range(NCORES):
        lo, hi = c * per_core, min((c + 1) * per_core, n_nodes)
        plans.append(_plan_core(lo, hi, dst_t, srcslot_t, counts_t, CAPS_T))
    nch = max(len(p) for p in plans)
    nch += (-nch) % G_CH
    ngroups = nch // G_CH

    sorted_t = []
    for t in range(ntyp):
        o = np.argsort(dst_t[t], kind="stable")
        sorted_t.append((dst_t[t][o], [srcslot_t[t][s][o] for s in range(t + 1)]))
    streams = [_build_streams(plans[c], nch, dst_t, srcslot_t, counts_t, bank_sizes,
                              sorted_t) for c in range(NCORES)]

    per_chunk_bank = [sum(SLOT_CAPS[si][b] for si in range(NSLOT)) * P for b in range(4)]
    bank_base = [[sum(SLOT_CAPS[sj][b] for sj in range(si)) * P for si in range(NSLOT)]
                 for b in range(4)]
    bank_region = [G_CH * per_chunk_bank[b] for b in range(4)]
    bank_tiles = [r // P for r in bank_region]

    x_pad = np.vstack([x, np.zeros((P, D), np.float32)])
    x_bf = x_pad.astype(ml_dtypes.bfloat16)
    iota = np.tile(np.arange(P, dtype=np.float32), (P, 1))
    ones_row = np.ones((1, P), np.float32)

    # ---- single u8 blob param for all per-core streams (fewer PJRT params =
    # fewer per-array axon transfers); 256B-aligned sections ----
    def _al(o):
        return (o + 255) & ~255
    sec = {}
    off = 0
    for b in range(nb_banks):
        nby = ngroups * 16 * (bank_region[b] // 16) * 2
        sec[f"idx{b}"] = (off, nby)
        off = _al(off + nby)
    nby = ngroups * P * G_CH * TILES_CHUNK
    sec["dst"] = (off, nby)
    off = _al(off + nby)
    nby = ngroups * P * G_CH * ntyp * 4
    sec["r"] = (off, nby)
    blob_bytes = _al(off + nby)

    # ---- build program ----
    nc = bacc.Bacc("TRN2", target_bir_lowering=False, debug=False,
                   num_devices=NCORES)
    dt = mybir.dt
    xs_d = nc.declare_dram_parameter("xs", [shard, D], dt.bfloat16, isOutput=False)
    blob_d = nc.declare_dram_parameter("blob", [blob_bytes], dt.uint8, isOutput=False)
    # output: int8 rows + per-row f32 scales packed in one u8 tensor
    out_qb = nch * P * D
    out_bytes = out_qb + nch * P * 4
    out_d = nc.declare_dram_parameter("out", [out_bytes], dt.uint8, isOutput=True)
    outq_ap = out_d[0:out_qb].rearrange(
        "(g c p d) -> g p c d", g=ngroups, c=G_CH, p=P)
    outs_ap = out_d[out_qb:out_bytes].bitcast(dt.float32).rearrange(
        "(g c p) -> g p c", g=ngroups, c=G_CH)

    def _sec_ap(name, dtype, pat, **axes):
        o, n = sec[name]
        ap = blob_d[o:o + n]
        if dtype != dt.uint8:
            ap = ap.bitcast(dtype)
        return ap.rearrange(pat, **axes)

    idx_ap = [_sec_ap(f"idx{b}", dt.int16, "(g p k) -> g p k", g=ngroups, p=16)
              for b in range(nb_banks)]
    dst_ap = _sec_ap("dst", dt.uint8, "(g p c k) -> g p c k",
                     g=ngroups, p=P, c=G_CH)
    r_ap = _sec_ap("r", dt.float32, "(g p c k) -> g p c k",
                   g=ngroups, p=P, c=G_CH)

    # shared constants ride in the NEFF (Const tensors), not over the wire
    w_c = nc.inline_tensor(np.ascontiguousarray(W_slots.transpose(1, 0, 2)), "wconst")
    io_c = nc.inline_tensor(iota, "ioconst")
    on_c = nc.inline_tensor(ones_row, "onconst")
    bc_c = nc.inline_tensor(bC.reshape(1, D).astype(np.float32), "bcconst")

    AF = mybir.ActivationFunctionType
    AL = mybir.AluOpType

    with tile.TileContext(nc) as tc:
        with (
            tc.tile_pool(name="const", bufs=1) as cpool,
            tc.tile_pool(name="dram", bufs=1, space="DRAM") as dram,
            tc.tile_pool(name="sbuf", bufs=2) as sb,
            tc.tile_pool(name="psum", bufs=2, space="PSUM") as ps,
        ):
            # x: shard -> bounce -> AllGather into full bf16 scratch
            x_in = dram.tile([shard, D], dt.bfloat16)
            x_full = dram.tile([n_pad, D], dt.bfloat16)
            nc.gpsimd.dma_start(out=x_in[:], in_=xs_d[:])
            nc.gpsimd.collective_compute(
                "AllGather", AL.bypass,
                replica_groups=[list(range(NCORES))],
                ins=[x_in[:].opt()], outs=[x_full[:].opt()])

            w_t = cpool.tile([P, NSLOT, D], dt.float32)
            nc.sync.dma_start(out=w_t[:], in_=w_c[:])
            io_t = cpool.tile([P, P], dt.float32)
            nc.sync.dma_start(out=io_t[:], in_=io_c[:])
            on_t = cpool.tile([1, P], dt.float32)
            nc.sync.dma_start(out=on_t[:], in_=on_c[:])
            bc_t = cpool.tile([1, P], dt.float32)
            nc.sync.dma_start(out=bc_t[:], in_=bc_c[:])

            with tc.For_i(0, ngroups) as gv:
                gtiles = []
                for b in range(nb_banks):
                    gt = sb.tile([P, bank_tiles[b], D], dt.bfloat16, tag=f"g{b}")
                    it = sb.tile([P, bank_region[b] // 16], dt.int16, tag=f"i{b}")
                    nc.sync.dma_start(out=it[0:16, :], in_=idx_ap[b][gv])
                    nc.sync.dma_start(out=it[16:32, :], in_=it[0:16, :])
                    nc.sync.dma_start(out=it[32:64, :], in_=it[0:32, :])
                    nc.sync.dma_start(out=it[64:128, :], in_=it[0:64, :])
                    if STAGE < 1 or bank_sizes[b] == 0:
                        nc.gpsimd.memset(gt[:], 0.0)
                        gtiles.append(gt)
                        continue
                    GMAX = 1024
                    for off in range(0, bank_region[b], GMAX):
                        n = min(GMAX, bank_region[b] - off)
                        nc.gpsimd.dma_gather(
                            out_ap=gt[:, off // P:(off + n) // P, :],
                            in_ap=x_full[b * BANK: b * BANK + bank_sizes[b], :],
                            idxs_ap=it[:, off // 16:(off + n) // 16],
                            num_idxs=n, num_idxs_reg=n, elem_size=D)
                    gtiles.append(gt)
                dst_u8 = sb.tile([P, G_CH, TILES_CHUNK], dt.uint8, tag="dst8")
                nc.sync.dma_start(out=dst_u8[:], in_=dst_ap[gv])
                dst_tl = sb.tile([P, G_CH, TILES_CHUNK], dt.float32, tag="dst")
                nc.vector.tensor_copy(out=dst_tl[:], in_=dst_u8[:])
                r_tl = sb.tile([P, G_CH, ntyp], dt.float32, tag="r")
                nc.sync.dma_start(out=r_tl[:], in_=r_ap[gv])
                out_tl = sb.tile([P, G_CH, D], dt.float32, tag="out")
                q8_tl = sb.tile([P, G_CH, D], dt.uint8, tag="q8")
                rmax_tl = sb.tile([P, G_CH, 1], dt.float32, tag="rmax")
                s_tl = sb.tile([P, G_CH], dt.float32, tag="sc")
                m_tl = sb.tile([P, G_CH], dt.float32, tag="mt")

                for cig in range(G_CH):
                    if STAGE < 2:
                        nc.vector.tensor_copy(out=out_tl[:, cig, :], in_=io_t[:])
                        continue
                    # R build: one DVE op per slot over its tiles
                    rt_tiles = {}
                    for si in range(NSLOT):
                        nt = SLOT_TILES[si]
                        rt = sb.tile([P, nt, P], dt.bfloat16, tag=f"R{si}")
                        nc.vector.tensor_tensor(
                            out=rt[:],
                            in0=dst_tl[:, cig, TILE_OFF[si]:TILE_OFF[si] + nt, None]
                                .to_broadcast([P, nt, P]),
                            in1=io_t[:, None, :].to_broadcast([P, nt, P]),
                            op=AL.is_equal)
                        rt_tiles[si] = rt
                    if STAGE < 3:
                        pass
                    # H accumulation
                    h_ps_a = ps.tile([P, 4 * P], dt.float32, space="PSUM", tag="ha")
                    h_ps_b = ps.tile([P, 3 * P], dt.float32, space="PSUM", tag="hb")
                    hmap = {}
                    for si in range(NSLOT):
                        if si < 4:
                            hmap[si] = h_ps_a[:, si * P:(si + 1) * P]
                        else:
                            hmap[si] = h_ps_b[:, (si - 4) * P:(si - 3) * P]
                    if STAGE < 3:
                        nc.vector.tensor_copy(out=out_tl[:, cig, :], in_=rt_tiles[0][:, 0, :])
                        continue
                    # one accumulation group per PSUM bank (start zeroes 2KB bank)
                    mm_a = []  # (out_slice, lhsT, rhs) for bank a (slots 0-3)
                    mm_b = []  # bank b (slots 4,5,6)
                    for si in range(NSLOT):
                        k = 0
                        for b in range(nb_banks):
                            base_t = (bank_base[b][si] + cig * per_chunk_bank[b]) // P
                            for tb in range(SLOT_CAPS[si][b]):
                                trip = (hmap[si], gtiles[b][:, base_t + tb, :],
                                        rt_tiles[si][:, k, :])
                                (mm_a if si < 4 else mm_b).append(trip)
                                k += 1
                    for mms in (mm_a, mm_b):
                        for i, (o, l, rr_) in enumerate(mms):
                            nc.tensor.matmul(out=o, lhsT=l, rhs=rr_,
                                             start=(i == 0), stop=(i == len(mms) - 1))
                    if STAGE < 4:
                        nc.scalar.activation(out=out_tl[:, cig, :], in_=h_ps_a[:, 0:P], func=AF.Copy)
                        continue
                    h_sb_a = sb.tile([P, 4 * P], dt.float32, tag="hsa")
                    nc.scalar.activation(out=h_sb_a[:], in_=h_ps_a[:], func=AF.Copy)
                    h_sb_b = sb.tile([P, 3 * P], dt.float32, tag="hsb")
                    nc.scalar.activation(out=h_sb_b[:], in_=h_ps_b[:], func=AF.Copy)
                    hs = {}
                    for si in range(NSLOT):
                        if si < 4:
                            hs[si] = h_sb_a[:, si * P:(si + 1) * P]
                        else:
                            hs[si] = h_sb_b[:, (si - 4) * P:(si - 3) * P]
                    # agg psum: [t0, t1, t2, self]
                    agg = ps.tile([P, 4 * P], dt.float32, space="PSUM", tag="agg")
                    mm_g = [(agg[:, 3 * P:4 * P], on_t[:], bc_t[:]),
                            (agg[:, 3 * P:4 * P], hs[NSLOT - 1], w_t[:, NSLOT - 1, :])]
                    slot_of_type = {0: [0], 1: [1, 2], 2: [3, 4, 5]}
                    for t in range(ntyp):
                        for si in slot_of_type[t]:
                            mm_g.append((agg[:, t * P:(t + 1) * P], hs[si], w_t[:, si, :]))
                    for i, (o, l, rr_) in enumerate(mm_g):
                        nc.tensor.matmul(out=o, lhsT=l, rhs=rr_,
                                         start=(i == 0), stop=(i == len(mm_g) - 1))
                    # combine: out = self + sum_t r_t * agg_t  (one PSUM input per op)
                    nc.scalar.activation(out=out_tl[:, cig, :], in_=agg[:, 3 * P:4 * P],
                                         func=AF.Copy)
                    for t in range(0, ntyp):
                        nc.vector.scalar_tensor_tensor(
                            out=out_tl[:, cig, :], in0=agg[:, t * P:(t + 1) * P],
                            scalar=r_tl[:, cig, t:t + 1], in1=out_tl[:, cig, :],
                            op0=AL.mult, op1=AL.add)
                # int8 quantization with per-row scale (host dequantizes)
                nc.vector.tensor_reduce(out=rmax_tl[:], in_=out_tl[:],
                                        axis=mybir.AxisListType.X,
                                        op=AL.max, apply_absolute_value=True)
                nc.vector.tensor_scalar(out=s_tl[:], in0=rmax_tl[:, :, 0],
                                        scalar1=1e-20, scalar2=1.0 / 126.5,
                                        op0=AL.max, op1=AL.mult)
                nc.vector.reciprocal(out=m_tl[:], in_=s_tl[:])
                for cig in range(G_CH):
                    # u8 = out/s + 127.5 in [1,254]; trunc==floor==round-half
                    nc.vector.tensor_scalar(
                        out=q8_tl[:, cig, :], in0=out_tl[:, cig, :],
                        scalar1=m_tl[:, cig:cig + 1], scalar2=127.5,
                        op0=AL.mult, op1=AL.add)
                nc.sync.dma_start(out=outq_ap[gv], in_=q8_tl[:])
                nc.sync.dma_start(out=outs_ap[gv], in_=s_tl[:])
    nc.finalize()

    in_maps = []
    for c in range(NCORES):
        idx_streams, dst_stream, r_arr, selfbase, meta = streams[c]
        blob = np.zeros(blob_bytes, np.uint8)
        for b in range(nb_banks):
            o, n = sec[f"idx{b}"]
            pk = np.stack([_wrap16(idx_streams[b][g]) for g in range(ngroups)])
            blob[o:o + n] = pk.view(np.uint8).ravel()
        o, n = sec["dst"]
        blob[o:o + n] = (dst_stream.reshape(ngroups, G_CH, TILES_CHUNK, P)
                         .transpose(0, 3, 1, 2).reshape(-1))
        o, n = sec["r"]
        blob[o:o + n] = (np.ascontiguousarray(
            r_arr.reshape(ngroups, G_CH, ntyp, P).transpose(0, 3, 1, 2))
            .view(np.uint8).ravel())
        in_maps.append(dict(xs=np.ascontiguousarray(x_bf[c * shard:(c + 1) * shard]),
                            blob=blob))

    global _LAST_NC, _LAST_INMAPS
    _LAST_NC, _LAST_INMAPS = nc, in_maps
    if sim:
        from concourse import bass_interp
        s = bass_interp.MultiCoreSim(nc, NCORES)
        for c in range(NCORES):
            for k, v in in_maps[c].items():
                s.cores[c].tensor(k)[:] = v
        s.simulate()
        results = [{"out": np.asarray(s.cores[c].tensor("out")).copy()}
                   for c in range(NCORES)]
        rr = type("R", (), {})(); rr.results = results; rr.exec_time_ns = None
        rr.sim = s
    else:
        import time as _time
        rr = run_bass_kernel_spmd(nc, in_maps, core_ids=list(range(NCORES)))
        if os.environ.get("KBENCH", "0") == "1":
            t0 = _time.time()
            rr = run_bass_kernel_spmd(nc, in_maps, core_ids=list(range(NCORES)))
            t1 = _time.time()
            print(f"warm call wall: {(t1-t0)*1e3:.1f} ms")
            t0 = _time.time()
            rr = run_bass_kernel_spmd(nc, in_maps, core_ids=list(range(NCORES)))
            t1 = _time.time()
            print(f"warm call 2 wall: {(t1-t0)*1e3:.1f} ms")
            print(f"HW exec time: {int((t1-t0)*1e9)} ns")

    out_qb = nch * P * D
    out_full = np.zeros((n_nodes, D), np.float32)
    for c in range(NCORES):
        _, _, _, _, meta = streams[c]
        buf = np.asarray(rr.results[c]["out"]).view(np.uint8).ravel()
        q = buf[:out_qb].reshape(nch, P, D).astype(np.float32) - 127.0
        s = buf[out_qb:out_qb + nch * P * 4].view(np.float32).reshape(nch, P)
        o = q * s[:, :, None]
        for ci, (lo, hi) in enumerate(meta):
            if hi > lo:
                out_full[lo:hi] = o[ci, :hi - lo]
    return out_full, rr


def kernel(x, src0, dst0, src1, dst1, src2, dst2, WA0, WA1, WA2, WC, bC):
    x = np.asarray(x, np.float32)
    n_nodes = x.shape[0]
    dst_t = [np.asarray(d, np.int64) for d in (dst0, dst1, dst2)]
    srcs = [np.asarray(s, np.int64) for s in (src0, src1, src2)]
    srcslot_t = [[srcs[t].reshape(-1, t + 1)[:, s] for s in range(t + 1)]
                 for t in range(3)]
    W_slots = np.stack([
        np.asarray(WA0, np.float32)[0:P],
        np.asarray(WA1, np.float32)[0:P], np.asarray(WA1, np.float32)[P:2 * P],
        np.asarray(WA2, np.float32)[0:P], np.asarray(WA2, np.float32)[P:2 * P],
        np.asarray(WA2, np.float32)[2 * P:3 * P],
        np.asarray(WC, np.float32).T.copy(),
    ])
    out, _ = _run(x, dst_t, srcslot_t, W_slots, None, np.asarray(bC, np.float32),
                  n_nodes)
    return out


# revision 4
# speedup vs baseline: 2.0025x; 1.2203x over previous
"""HGNN layer kernel for 8 Trainium2 NeuronCores.

Strategy: shard by destination node. Host cuts the node range into contiguous
variable-size chunks (<=128 nodes, per-type/slot/bank edge caps), assigns an
equal number of chunks to each core (uniform SPMD program). x is shipped
SHARDED (1/8th per core, bf16) and AllGathered on-device into a DRAM scratch
to keep the host->device wire traffic minimal (the axon PJRT tunnel is the
wall-clock bottleneck, ~35 MB/s). Per chunk, each edge-type/slot stream is
gathered from the scratch via dma_gather (4 high-bit banks so indices fit
int16; index tables ship 16-partition-packed and are replicated to 128
partitions on device), then a one-hot selection matrix R (built on DVE from
u8 dst positions) turns gather+matmul+segment-sum into:
    H_s   = G_s.T @ R        (PE bf16, accumulated over the slot's tiles in PSUM)
    agg_t = sum_s H_s.T @ W_s  (PE f32)
    out   = sum_t r_t * agg_t + x@WC.T + bC   (DVE scalar_tensor_tensor)
Normalization r_t = 1/count is host-derived index metadata (like the CSR sort).
No output collectives needed: each core owns its chunks' outputs (bf16 on the
wire, f32 on host).
"""
import sys, os
sys.path.insert(0, "/opt/trn_rl_repo")
import numpy as np
import ml_dtypes
STAGE = int(os.environ.get("STAGE", "9"))  # 1=gathers only, 2=+R, 3=+H, 4=+agg, 9=all

P = 128
D = 128
NCORES = 8
BANK = 32768
CAPS_T = (2, 2, 2, 1)          # tiles per bank segment (bank3 is the 1696-row tail)
CAPS_SELF = (1, 1, 1, 1)
SLOTS = ((0, 0), (1, 0), (1, 1), (2, 0), (2, 1), (2, 2), (3, 0))  # (type, slot); 3 = self
NSLOT = len(SLOTS)              # 6 edge slots + self
SLOT_CAPS = [CAPS_T] * 6 + [CAPS_SELF]
SLOT_TILES = [sum(c) for c in SLOT_CAPS]
TILES_CHUNK = sum(SLOT_TILES)   # 46
TILE_OFF = np.cumsum([0] + SLOT_TILES).tolist()
G_CH = 2                        # chunks per pipeline group
DST_PAD = 255                   # u8 pad marker (>= P, matches no iota value)


def _plan_core(node_lo, node_hi, dst_t, srcslot_t, counts_t, caps):
    """Cut [node_lo, node_hi) into chunks and build per-chunk streams."""
    nodes = np.arange(node_lo, node_hi)
    # per-node per-(t,s,b) edge counts for the cutting pass
    percnt = np.zeros((node_hi - node_lo, 6, 4), np.int32)
    for si, (t, s) in enumerate(SLOTS[:6]):
        dst = dst_t[t]
        sel = (dst >= node_lo) & (dst < node_hi)
        b = (srcslot_t[t][s][sel] >> 15).astype(np.int64)
        np.add.at(percnt, (dst[sel] - node_lo, si, np.minimum(b, 3)), 1)
    chunks = []
    i, n = 0, node_hi - node_lo
    segcap = np.array(caps, np.int32) * P
    while i < n:
        acc = np.zeros((6, 4), np.int32)
        j = i
        while j < n and j - i < P:
            nxt = acc + percnt[j]
            if (nxt > segcap[None, :]).any():
                break
            acc = nxt
            j += 1
        if j == i:  # single node exceeding a cap: shouldn't happen at this scale
            j = i + 1
        chunks.append((node_lo + i, node_lo + j))
        i = j
    return chunks


def _build_streams(chunks, nch, dst_t, srcslot_t, counts_t, bank_sizes, sorted_t=None):
    """Per-core stream arrays for the uniform program."""
    ntyp = len(dst_t)
    # index streams per bank (G order: group-major, bank-major inside group)
    ngroups = nch // G_CH
    # within bank b's region (per group): per chunk, slots in order, each cap[si][b]*P
    per_chunk_bank = [sum(SLOT_CAPS[si][b] for si in range(NSLOT)) * P for b in range(4)]
    bank_base = [[sum(SLOT_CAPS[sj][b] for sj in range(si)) * P for si in range(NSLOT)]
                 for b in range(4)]
    bank_region = [G_CH * per_chunk_bank[b] for b in range(4)]
    idx_streams = [np.zeros((ngroups, bank_region[b]), np.int16) for b in range(4)]
    # dst stream (R order: chunk-major; per chunk: slots, then bank segs in order)
    dst_stream = np.full((nch, TILES_CHUNK * P), DST_PAD, np.uint8)
    r_arr = np.zeros((nch, ntyp, P), np.float32)
    selfbase = np.zeros(nch, np.int32)
    meta = []
    for ci in range(nch):
        if ci < len(chunks):
            lo, hi = chunks[ci]
        else:
            lo, hi = 0, 0  # empty pad chunk; selfbase points at pad rows
        meta.append((lo, hi))
        selfbase[ci] = lo if hi > lo else 0
        g, cig = ci // G_CH, ci % G_CH
        for si, (t, s) in enumerate(SLOTS):
            if t < 3:
                sdst, ssrc = sorted_t[t]
                a = np.searchsorted(sdst, lo)
                z = np.searchsorted(sdst, hi)
                e_dst = sdst[a:z] - lo
                e_src = ssrc[s][a:z]
            else:  # self slot: node -> its own position
                e_src = np.arange(lo, hi, dtype=np.int64)
                e_dst = np.arange(hi - lo, dtype=np.int64)
            order = np.argsort(e_src >> 15, kind="stable")
            e_dst, e_src = e_dst[order], e_src[order]
            bank = (e_src >> 15).astype(np.int64)
            dcol0 = TILE_OFF[si] * P
            seg_off = 0
            for b in range(4):
                m = bank == b
                sb = e_src[m] - b * BANK
                db = e_dst[m]
                nb = sb.shape[0]
                caps = SLOT_CAPS[si]
                assert nb <= caps[b] * P, (si, b, nb)
                base = bank_base[b][si] + cig * per_chunk_bank[b]
                idx_streams[b][g, base:base + nb] = sb.astype(np.int16)
                # pads keep 0 (gather bank row 0, dst stays DST_PAD)
                dst_stream[ci, dcol0 + seg_off: dcol0 + seg_off + nb] = db.astype(np.uint8)
                seg_off += caps[b] * P
        for t in range(ntyp):
            npos = hi - lo
            if npos > 0:
                c = counts_t[t][lo:hi].astype(np.float32)
                r = np.where(c > 0, 1.0 / np.maximum(c, 1.0), 0.0)
                r_arr[ci, t, :npos] = r
    return idx_streams, dst_stream, r_arr, selfbase, meta


def _wrap16(idx_flat):
    """dma_gather index layout: j -> [j%16, j//16]; device replicates to 128 parts."""
    n = idx_flat.shape[0]
    w = np.zeros((16, n // 16), np.int16)
    j = np.arange(n)
    w[j % 16, j // 16] = idx_flat
    return w


def _run(x, dst_t, srcslot_t, W_slots, WCt, bC, n_nodes, core_ids=None, sim=False):
    from concourse import bass, bacc, mybir, tile
    from concourse.bass_utils import run_bass_kernel_spmd

    ntyp = len(dst_t)
    counts_t = [np.bincount(dst_t[t], minlength=n_nodes) for t in range(ntyp)]
    bank_sizes = [min(BANK, max(0, n_nodes - b * BANK)) for b in range(4)]
    nb_banks = sum(1 for s in bank_sizes if s > 0)
    n_pad = n_nodes + P
    # x ships as a u8 byte blob: bf16 rows + f32 weights/bias/ones appended,
    # sharded across cores and AllGathered on device (dedups the 8x W copy)
    XROW = D * 2
    w_off = n_pad * XROW
    bc_off = w_off + NSLOT * P * D * 4
    on_off = bc_off + D * 4
    io_off = on_off + D * 4
    x_bytes = io_off + P * P * 4
    x_bytes += (-x_bytes) % (NCORES * 256)
    assert x_bytes % NCORES == 0
    shard_b = x_bytes // NCORES

    # ---- per-core planning (uniform structure across cores) ----
    per_core = (n_nodes + NCORES - 1) // NCORES
    plans = []
    for c in range(NCORES):
        lo, hi = c * per_core, min((c + 1) * per_core, n_nodes)
        plans.append(_plan_core(lo, hi, dst_t, srcslot_t, counts_t, CAPS_T))
    nch = max(len(p) for p in plans)
    nch += (-nch) % G_CH
    ngroups = nch // G_CH

    sorted_t = []
    for t in range(ntyp):
        o = np.argsort(dst_t[t], kind="stable")
        sorted_t.append((dst_t[t][o], [srcslot_t[t][s][o] for s in range(t + 1)]))
    streams = [_build_streams(plans[c], nch, dst_t, srcslot_t, counts_t, bank_sizes,
                              sorted_t) for c in range(NCORES)]

    per_chunk_bank = [sum(SLOT_CAPS[si][b] for si in range(NSLOT)) * P for b in range(4)]
    bank_base = [[sum(SLOT_CAPS[sj][b] for sj in range(si)) * P for si in range(NSLOT)]
                 for b in range(4)]
    bank_region = [G_CH * per_chunk_bank[b] for b in range(4)]
    bank_tiles = [r // P for r in bank_region]

    x_pad = np.vstack([x, np.zeros((P, D), np.float32)])
    x_bf = x_pad.astype(ml_dtypes.bfloat16)
    iota = np.tile(np.arange(P, dtype=np.float32), (P, 1))
    ones_row = np.ones((1, P), np.float32)
    wt_f32 = np.ascontiguousarray(W_slots.transpose(1, 0, 2)).astype(np.float32)

    # ---- single u8 blob param for all per-core streams (fewer PJRT params =
    # fewer per-array axon transfers); 256B-aligned sections ----
    def _al(o):
        return (o + 255) & ~255
    sec = {}
    off = 0
    for b in range(nb_banks):
        nby = ngroups * 16 * (bank_region[b] // 16) * 2
        sec[f"idx{b}"] = (off, nby)
        off = _al(off + nby)
    nby = ngroups * P * G_CH * TILES_CHUNK
    sec["dst"] = (off, nby)
    off = _al(off + nby)
    nby = ngroups * P * G_CH * ntyp * 4
    sec["r"] = (off, nby)
    blob_bytes = _al(off + nby)

    # ---- build program ----
    nc = bacc.Bacc("TRN2", target_bir_lowering=False, debug=False,
                   num_devices=NCORES)
    dt = mybir.dt
    xs_d = nc.declare_dram_parameter("xs", [shard_b], dt.uint8, isOutput=False)
    blob_d = nc.declare_dram_parameter("blob", [blob_bytes], dt.uint8, isOutput=False)
    # output: int8 rows + per-row f32 scales packed in one u8 tensor
    out_qb = nch * P * D
    out_bytes = out_qb + nch * P * 4
    out_d = nc.declare_dram_parameter("out", [out_bytes], dt.uint8, isOutput=True)
    outq_ap = out_d[0:out_qb].rearrange(
        "(g c p d) -> g p c d", g=ngroups, c=G_CH, p=P)
    outs_ap = out_d[out_qb:out_bytes].bitcast(dt.float32).rearrange(
        "(g c p) -> g p c", g=ngroups, c=G_CH)

    def _sec_ap(name, dtype, pat, **axes):
        o, n = sec[name]
        ap = blob_d[o:o + n]
        if dtype != dt.uint8:
            ap = ap.bitcast(dtype)
        return ap.rearrange(pat, **axes)

    idx_ap = [_sec_ap(f"idx{b}", dt.int16, "(g p k) -> g p k", g=ngroups, p=16)
              for b in range(nb_banks)]
    dst_ap = _sec_ap("dst", dt.uint8, "(g p c k) -> g p c k",
                     g=ngroups, p=P, c=G_CH)
    r_ap = _sec_ap("r", dt.float32, "(g p c k) -> g p c k",
                   g=ngroups, p=P, c=G_CH)

    AFx = None  # (constants now ride the x AllGather; iota is generated on-device)

    AF = mybir.ActivationFunctionType
    AL = mybir.AluOpType

    with tile.TileContext(nc) as tc:
        with (
            tc.tile_pool(name="const", bufs=1) as cpool,
            tc.tile_pool(name="dram", bufs=1, space="DRAM") as dram,
            tc.tile_pool(name="sbuf", bufs=2) as sb,
            tc.tile_pool(name="psum", bufs=2, space="PSUM") as ps,
        ):
            # x: shard -> bounce -> AllGather into full byte scratch
            x_in = dram.tile([shard_b], dt.uint8)
            x_full = dram.tile([x_bytes], dt.uint8)
            nc.gpsimd.dma_start(out=x_in[:], in_=xs_d[:])
            nc.gpsimd.collective_compute(
                "AllGather", AL.bypass,
                replica_groups=[list(range(NCORES))],
                ins=[x_in[:].opt()], outs=[x_full[:].opt()])
            x_rows = x_full[0:n_pad * XROW].bitcast(dt.bfloat16).rearrange(
                "(n d) -> n d", d=D)

            w_t = cpool.tile([P, NSLOT, D], dt.float32)
            nc.sync.dma_start(
                out=w_t[:],
                in_=x_full[w_off:w_off + NSLOT * P * D * 4].bitcast(dt.float32)
                .rearrange("(p w d) -> p w d", w=NSLOT, d=D))
            io_t = cpool.tile([P, P], dt.float32)
            nc.sync.dma_start(
                out=io_t[:],
                in_=x_full[io_off:io_off + P * P * 4].bitcast(dt.float32)
                .rearrange("(p d) -> p d", d=P))
            on_t = cpool.tile([1, P], dt.float32)
            nc.sync.dma_start(
                out=on_t[:],
                in_=x_full[on_off:on_off + D * 4].bitcast(dt.float32)
                .rearrange("(a d) -> a d", a=1))
            bc_t = cpool.tile([1, P], dt.float32)
            nc.sync.dma_start(
                out=bc_t[:],
                in_=x_full[bc_off:bc_off + D * 4].bitcast(dt.float32)
                .rearrange("(a d) -> a d", a=1))

            with tc.For_i(0, ngroups) as gv:
                gtiles = []
                for b in range(nb_banks):
                    gt = sb.tile([P, bank_tiles[b], D], dt.bfloat16, tag=f"g{b}")
                    it = sb.tile([P, bank_region[b] // 16], dt.int16, tag=f"i{b}")
                    nc.sync.dma_start(out=it[0:16, :], in_=idx_ap[b][gv])
                    nc.sync.dma_start(out=it[16:32, :], in_=it[0:16, :])
                    nc.sync.dma_start(out=it[32:64, :], in_=it[0:32, :])
                    nc.sync.dma_start(out=it[64:128, :], in_=it[0:64, :])
                    if STAGE < 1 or bank_sizes[b] == 0:
                        nc.gpsimd.memset(gt[:], 0.0)
                        gtiles.append(gt)
                        continue
                    GMAX = 1024
                    for off in range(0, bank_region[b], GMAX):
                        n = min(GMAX, bank_region[b] - off)
                        nc.gpsimd.dma_gather(
                            out_ap=gt[:, off // P:(off + n) // P, :],
                            in_ap=x_rows[b * BANK: b * BANK + bank_sizes[b], :],
                            idxs_ap=it[:, off // 16:(off + n) // 16],
                            num_idxs=n, num_idxs_reg=n, elem_size=D)
                    gtiles.append(gt)
                dst_u8 = sb.tile([P, G_CH, TILES_CHUNK], dt.uint8, tag="dst8")
                nc.sync.dma_start(out=dst_u8[:], in_=dst_ap[gv])
                dst_tl = sb.tile([P, G_CH, TILES_CHUNK], dt.float32, tag="dst")
                nc.vector.tensor_copy(out=dst_tl[:], in_=dst_u8[:])
                r_tl = sb.tile([P, G_CH, ntyp], dt.float32, tag="r")
                nc.sync.dma_start(out=r_tl[:], in_=r_ap[gv])
                out_tl = sb.tile([P, G_CH, D], dt.float32, tag="out")
                q8_tl = sb.tile([P, G_CH, D], dt.uint8, tag="q8")
                rmax_tl = sb.tile([P, G_CH, 1], dt.float32, tag="rmax")
                s_tl = sb.tile([P, G_CH], dt.float32, tag="sc")
                m_tl = sb.tile([P, G_CH], dt.float32, tag="mt")

                for cig in range(G_CH):
                    if STAGE < 2:
                        nc.vector.tensor_copy(out=out_tl[:, cig, :], in_=io_t[:])
                        continue
                    # R build: one DVE op per slot over its tiles
                    rt_tiles = {}
                    for si in range(NSLOT):
                        nt = SLOT_TILES[si]
                        rt = sb.tile([P, nt, P], dt.bfloat16, tag=f"R{si}")
                        nc.vector.tensor_tensor(
                            out=rt[:],
                            in0=dst_tl[:, cig, TILE_OFF[si]:TILE_OFF[si] + nt, None]
                                .to_broadcast([P, nt, P]),
                            in1=io_t[:, None, :].to_broadcast([P, nt, P]),
                            op=AL.is_equal)
                        rt_tiles[si] = rt
                    if STAGE < 3:
                        pass
                    # H accumulation
                    h_ps_a = ps.tile([P, 4 * P], dt.float32, space="PSUM", tag="ha")
                    h_ps_b = ps.tile([P, 3 * P], dt.float32, space="PSUM", tag="hb")
                    hmap = {}
                    for si in range(NSLOT):
                        if si < 4:
                            hmap[si] = h_ps_a[:, si * P:(si + 1) * P]
                        else:
                            hmap[si] = h_ps_b[:, (si - 4) * P:(si - 3) * P]
                    if STAGE < 3:
                        nc.vector.tensor_copy(out=out_tl[:, cig, :], in_=rt_tiles[0][:, 0, :])
                        continue
                    # one accumulation group per PSUM bank (start zeroes 2KB bank)
                    mm_a = []  # (out_slice, lhsT, rhs) for bank a (slots 0-3)
                    mm_b = []  # bank b (slots 4,5,6)
                    for si in range(NSLOT):
                        k = 0
                        for b in range(nb_banks):
                            base_t = (bank_base[b][si] + cig * per_chunk_bank[b]) // P
                            for tb in range(SLOT_CAPS[si][b]):
                                trip = (hmap[si], gtiles[b][:, base_t + tb, :],
                                        rt_tiles[si][:, k, :])
                                (mm_a if si < 4 else mm_b).append(trip)
                                k += 1
                    for mms in (mm_a, mm_b):
                        for i, (o, l, rr_) in enumerate(mms):
                            nc.tensor.matmul(out=o, lhsT=l, rhs=rr_,
                                             start=(i == 0), stop=(i == len(mms) - 1))
                    if STAGE < 4:
                        nc.scalar.activation(out=out_tl[:, cig, :], in_=h_ps_a[:, 0:P], func=AF.Copy)
                        continue
                    h_sb_a = sb.tile([P, 4 * P], dt.float32, tag="hsa")
                    nc.scalar.activation(out=h_sb_a[:], in_=h_ps_a[:], func=AF.Copy)
                    h_sb_b = sb.tile([P, 3 * P], dt.float32, tag="hsb")
                    nc.scalar.activation(out=h_sb_b[:], in_=h_ps_b[:], func=AF.Copy)
                    hs = {}
                    for si in range(NSLOT):
                        if si < 4:
                            hs[si] = h_sb_a[:, si * P:(si + 1) * P]
                        else:
                            hs[si] = h_sb_b[:, (si - 4) * P:(si - 3) * P]
                    # agg psum: [t0, t1, t2, self]
                    agg = ps.tile([P, 4 * P], dt.float32, space="PSUM", tag="agg")
                    mm_g = [(agg[:, 3 * P:4 * P], on_t[:], bc_t[:]),
                            (agg[:, 3 * P:4 * P], hs[NSLOT - 1], w_t[:, NSLOT - 1, :])]
                    slot_of_type = {0: [0], 1: [1, 2], 2: [3, 4, 5]}
                    for t in range(ntyp):
                        for si in slot_of_type[t]:
                            mm_g.append((agg[:, t * P:(t + 1) * P], hs[si], w_t[:, si, :]))
                    for i, (o, l, rr_) in enumerate(mm_g):
                        nc.tensor.matmul(out=o, lhsT=l, rhs=rr_,
                                         start=(i == 0), stop=(i == len(mm_g) - 1))
                    # combine: out = self + sum_t r_t * agg_t  (one PSUM input per op)
                    nc.scalar.activation(out=out_tl[:, cig, :], in_=agg[:, 3 * P:4 * P],
                                         func=AF.Copy)
                    for t in range(0, ntyp):
                        nc.vector.scalar_tensor_tensor(
                            out=out_tl[:, cig, :], in0=agg[:, t * P:(t + 1) * P],
                            scalar=r_tl[:, cig, t:t + 1], in1=out_tl[:, cig, :],
                            op0=AL.mult, op1=AL.add)
                # int8 quantization with per-row scale (host dequantizes)
                nc.vector.tensor_reduce(out=rmax_tl[:], in_=out_tl[:],
                                        axis=mybir.AxisListType.X,
                                        op=AL.max, apply_absolute_value=True)
                nc.vector.tensor_scalar(out=s_tl[:], in0=rmax_tl[:, :, 0],
                                        scalar1=1e-20, scalar2=1.0 / 126.5,
                                        op0=AL.max, op1=AL.mult)
                nc.vector.reciprocal(out=m_tl[:], in_=s_tl[:])
                for cig in range(G_CH):
                    # u8 = out/s + 127.5 in [1,254]; trunc==floor==round-half
                    nc.vector.tensor_scalar(
                        out=q8_tl[:, cig, :], in0=out_tl[:, cig, :],
                        scalar1=m_tl[:, cig:cig + 1], scalar2=127.5,
                        op0=AL.mult, op1=AL.add)
                nc.sync.dma_start(out=outq_ap[gv], in_=q8_tl[:])
                nc.sync.dma_start(out=outs_ap[gv], in_=s_tl[:])
    nc.finalize()

    x_aug = np.zeros(x_bytes, np.uint8)
    x_aug[0:n_pad * XROW] = np.frombuffer(x_bf.tobytes(), np.uint8)
    x_aug[w_off:w_off + wt_f32.nbytes] = np.frombuffer(wt_f32.tobytes(), np.uint8)
    x_aug[bc_off:bc_off + D * 4] = np.frombuffer(
        bC.reshape(1, D).astype(np.float32).tobytes(), np.uint8)
    x_aug[on_off:on_off + D * 4] = np.frombuffer(ones_row.tobytes(), np.uint8)
    x_aug[io_off:io_off + P * P * 4] = np.frombuffer(iota.tobytes(), np.uint8)

    in_maps = []
    for c in range(NCORES):
        idx_streams, dst_stream, r_arr, selfbase, meta = streams[c]
        blob = np.zeros(blob_bytes, np.uint8)
        for b in range(nb_banks):
            o, n = sec[f"idx{b}"]
            pk = np.stack([_wrap16(idx_streams[b][g]) for g in range(ngroups)])
            blob[o:o + n] = pk.view(np.uint8).ravel()
        o, n = sec["dst"]
        blob[o:o + n] = (dst_stream.reshape(ngroups, G_CH, TILES_CHUNK, P)
                         .transpose(0, 3, 1, 2).reshape(-1))
        o, n = sec["r"]
        blob[o:o + n] = (np.ascontiguousarray(
            r_arr.reshape(ngroups, G_CH, ntyp, P).transpose(0, 3, 1, 2))
            .view(np.uint8).ravel())
        in_maps.append(dict(xs=x_aug[c * shard_b:(c + 1) * shard_b].copy(),
                            blob=blob))

    global _LAST_NC, _LAST_INMAPS
    _LAST_NC, _LAST_INMAPS = nc, in_maps
    if sim:
        from concourse import bass_interp
        s = bass_interp.MultiCoreSim(nc, NCORES)
        for c in range(NCORES):
            for k, v in in_maps[c].items():
                s.cores[c].tensor(k)[:] = v
        s.simulate()
        results = [{"out": np.asarray(s.cores[c].tensor("out")).copy()}
                   for c in range(NCORES)]
        rr = type("R", (), {})(); rr.results = results; rr.exec_time_ns = None
        rr.sim = s
    else:
        import time as _time
        rr = run_bass_kernel_spmd(nc, in_maps, core_ids=list(range(NCORES)))
        if os.environ.get("KBENCH", "0") == "1":
            t0 = _time.time()
            rr = run_bass_kernel_spmd(nc, in_maps, core_ids=list(range(NCORES)))
            t1 = _time.time()
            print(f"warm call wall: {(t1-t0)*1e3:.1f} ms")
            t0 = _time.time()
            rr = run_bass_kernel_spmd(nc, in_maps, core_ids=list(range(NCORES)))
            t1 = _time.time()
            print(f"warm call 2 wall: {(t1-t0)*1e3:.1f} ms")
            print(f"HW exec time: {int((t1-t0)*1e9)} ns")

    out_qb = nch * P * D
    out_full = np.zeros((n_nodes, D), np.float32)
    for c in range(NCORES):
        _, _, _, _, meta = streams[c]
        buf = np.asarray(rr.results[c]["out"]).view(np.uint8).ravel()
        q = buf[:out_qb].reshape(nch, P, D).astype(np.float32) - 127.0
        s = buf[out_qb:out_qb + nch * P * 4].view(np.float32).reshape(nch, P)
        o = q * s[:, :, None]
        for ci, (lo, hi) in enumerate(meta):
            if hi > lo:
                out_full[lo:hi] = o[ci, :hi - lo]
    return out_full, rr


def kernel(x, src0, dst0, src1, dst1, src2, dst2, WA0, WA1, WA2, WC, bC):
    x = np.asarray(x, np.float32)
    n_nodes = x.shape[0]
    dst_t = [np.asarray(d, np.int64) for d in (dst0, dst1, dst2)]
    srcs = [np.asarray(s, np.int64) for s in (src0, src1, src2)]
    srcslot_t = [[srcs[t].reshape(-1, t + 1)[:, s] for s in range(t + 1)]
                 for t in range(3)]
    W_slots = np.stack([
        np.asarray(WA0, np.float32)[0:P],
        np.asarray(WA1, np.float32)[0:P], np.asarray(WA1, np.float32)[P:2 * P],
        np.asarray(WA2, np.float32)[0:P], np.asarray(WA2, np.float32)[P:2 * P],
        np.asarray(WA2, np.float32)[2 * P:3 * P],
        np.asarray(WC, np.float32).T.copy(),
    ])
    out, _ = _run(x, dst_t, srcslot_t, W_slots, None, np.asarray(bC, np.float32),
                  n_nodes)
    return out


# revision 6
# speedup vs baseline: 2.0098x; 1.0037x over previous
"""HGNN layer kernel for 8 Trainium2 NeuronCores.

Strategy: shard by destination node. Host cuts the node range into contiguous
variable-size chunks (<=128 nodes, per-type/slot/bank edge caps), assigns an
equal number of chunks to each core (uniform SPMD program). x is shipped
SHARDED (1/8th per core, bf16) and AllGathered on-device into a DRAM scratch
to keep the host->device wire traffic minimal (the axon PJRT tunnel is the
wall-clock bottleneck, ~35 MB/s). Per chunk, each edge-type/slot stream is
gathered from the scratch via dma_gather (4 high-bit banks so indices fit
int16; index tables ship 16-partition-packed and are replicated to 128
partitions on device), then a one-hot selection matrix R (built on DVE from
u8 dst positions) turns gather+matmul+segment-sum into:
    H_s   = G_s.T @ R        (PE bf16, accumulated over the slot's tiles in PSUM)
    agg_t = sum_s H_s.T @ W_s  (PE f32)
    out   = sum_t r_t * agg_t + x@WC.T + bC   (DVE scalar_tensor_tensor)
Normalization r_t = 1/count is host-derived index metadata (like the CSR sort).
No output collectives needed: each core owns its chunks' outputs (bf16 on the
wire, f32 on host).
"""
import sys, os
sys.path.insert(0, "/opt/trn_rl_repo")
import numpy as np
import ml_dtypes
STAGE = int(os.environ.get("STAGE", "9"))  # 1=gathers only, 2=+R, 3=+H, 4=+agg, 9=all

P = 128
D = 128
NCORES = 8
BANK = 32768
CAPS_T = (2, 2, 2, 1)          # tiles per bank segment (bank3 is the 1696-row tail)
CAPS_SELF = (1, 1, 1, 1)
SLOTS = ((0, 0), (1, 0), (1, 1), (2, 0), (2, 1), (2, 2), (3, 0))  # (type, slot); 3 = self
NSLOT = len(SLOTS)              # 6 edge slots + self
SLOT_CAPS = [CAPS_T] * 6 + [CAPS_SELF]
SLOT_TILES = [sum(c) for c in SLOT_CAPS]
TILES_CHUNK = sum(SLOT_TILES)   # 46
TILE_OFF = np.cumsum([0] + SLOT_TILES).tolist()
G_CH = 2                        # chunks per pipeline group
DST_PAD = 255                   # u8 pad marker (>= P, matches no iota value)


def _plan_core(node_lo, node_hi, dst_t, srcslot_t, counts_t, caps):
    """Cut [node_lo, node_hi) into chunks and build per-chunk streams."""
    nodes = np.arange(node_lo, node_hi)
    # per-node per-(t,s,b) edge counts for the cutting pass
    percnt = np.zeros((node_hi - node_lo, 6, 4), np.int32)
    for si, (t, s) in enumerate(SLOTS[:6]):
        dst = dst_t[t]
        sel = (dst >= node_lo) & (dst < node_hi)
        b = (srcslot_t[t][s][sel] >> 15).astype(np.int64)
        np.add.at(percnt, (dst[sel] - node_lo, si, np.minimum(b, 3)), 1)
    chunks = []
    i, n = 0, node_hi - node_lo
    segcap = np.array(caps, np.int32) * P
    while i < n:
        acc = np.zeros((6, 4), np.int32)
        j = i
        while j < n and j - i < P:
            nxt = acc + percnt[j]
            if (nxt > segcap[None, :]).any():
                break
            acc = nxt
            j += 1
        if j == i:  # single node exceeding a cap: shouldn't happen at this scale
            j = i + 1
        chunks.append((node_lo + i, node_lo + j))
        i = j
    return chunks


def _build_streams(chunks, nch, dst_t, srcslot_t, counts_t, bank_sizes, sorted_t=None):
    """Per-core stream arrays for the uniform program."""
    ntyp = len(dst_t)
    # index streams per bank (G order: group-major, bank-major inside group)
    ngroups = nch // G_CH
    # within bank b's region (per group): per chunk, slots in order, each cap[si][b]*P
    per_chunk_bank = [sum(SLOT_CAPS[si][b] for si in range(NSLOT)) * P for b in range(4)]
    bank_base = [[sum(SLOT_CAPS[sj][b] for sj in range(si)) * P for si in range(NSLOT)]
                 for b in range(4)]
    bank_region = [G_CH * per_chunk_bank[b] for b in range(4)]
    idx_streams = [np.zeros((ngroups, bank_region[b]), np.int16) for b in range(4)]
    # dst stream (R order: chunk-major; per chunk: slots, then bank segs in order)
    dst_stream = np.full((nch, TILES_CHUNK * P), DST_PAD, np.uint8)
    r_arr = np.zeros((nch, ntyp, P), np.float32)
    selfbase = np.zeros(nch, np.int32)
    meta = []
    for ci in range(nch):
        if ci < len(chunks):
            lo, hi = chunks[ci]
        else:
            lo, hi = 0, 0  # empty pad chunk; selfbase points at pad rows
        meta.append((lo, hi))
        selfbase[ci] = lo if hi > lo else 0
        g, cig = ci // G_CH, ci % G_CH
        for si, (t, s) in enumerate(SLOTS):
            if t < 3:
                sdst, ssrc = sorted_t[t]
                a = np.searchsorted(sdst, lo)
                z = np.searchsorted(sdst, hi)
                e_dst = sdst[a:z] - lo
                e_src = ssrc[s][a:z]
            else:  # self slot: node -> its own position
                e_src = np.arange(lo, hi, dtype=np.int64)
                e_dst = np.arange(hi - lo, dtype=np.int64)
            order = np.argsort(e_src >> 15, kind="stable")
            e_dst, e_src = e_dst[order], e_src[order]
            bank = (e_src >> 15).astype(np.int64)
            dcol0 = TILE_OFF[si] * P
            seg_off = 0
            for b in range(4):
                m = bank == b
                sb = e_src[m] - b * BANK
                db = e_dst[m]
                nb = sb.shape[0]
                caps = SLOT_CAPS[si]
                assert nb <= caps[b] * P, (si, b, nb)
                base = bank_base[b][si] + cig * per_chunk_bank[b]
                idx_streams[b][g, base:base + nb] = sb.astype(np.int16)
                # pads keep 0 (gather bank row 0, dst stays DST_PAD)
                dst_stream[ci, dcol0 + seg_off: dcol0 + seg_off + nb] = db.astype(np.uint8)
                seg_off += caps[b] * P
        for t in range(ntyp):
            npos = hi - lo
            if npos > 0:
                c = counts_t[t][lo:hi].astype(np.float32)
                r = np.where(c > 0, 1.0 / np.maximum(c, 1.0), 0.0)
                r_arr[ci, t, :npos] = r
    return idx_streams, dst_stream, r_arr, selfbase, meta


def _wrap16(idx_flat):
    """dma_gather index layout: j -> [j%16, j//16]; device replicates to 128 parts."""
    n = idx_flat.shape[0]
    w = np.zeros((16, n // 16), np.int16)
    j = np.arange(n)
    w[j % 16, j // 16] = idx_flat
    return w


def _run(x, dst_t, srcslot_t, W_slots, WCt, bC, n_nodes, core_ids=None, sim=False):
    from concourse import bass, bacc, mybir, tile
    from concourse.bass_utils import run_bass_kernel_spmd

    ntyp = len(dst_t)
    counts_t = [np.bincount(dst_t[t], minlength=n_nodes) for t in range(ntyp)]
    bank_sizes = [min(BANK, max(0, n_nodes - b * BANK)) for b in range(4)]
    nb_banks = sum(1 for s in bank_sizes if s > 0)
    n_pad = n_nodes + P
    # x ships as a u8 byte blob: per-row 128 int8 + bf16 row-scale (130B packed),
    # plus f32 weights/bias/ones/iota appended; sharded across cores and
    # AllGathered on device, then expanded to 256B-strided rows for dma_gather
    XROW = 130
    XG = 256
    w_off = n_pad * XROW
    w_off += (-w_off) % 4
    bc_off = w_off + NSLOT * P * D * 4
    on_off = bc_off + D * 4
    io_off = on_off + D * 4
    x_bytes = io_off + P * P * 4
    x_bytes += (-x_bytes) % (NCORES * 256)
    assert x_bytes % NCORES == 0
    shard_b = x_bytes // NCORES

    # ---- per-core planning (uniform structure across cores) ----
    per_core = (n_nodes + NCORES - 1) // NCORES
    plans = []
    for c in range(NCORES):
        lo, hi = c * per_core, min((c + 1) * per_core, n_nodes)
        plans.append(_plan_core(lo, hi, dst_t, srcslot_t, counts_t, CAPS_T))
    nch = max(len(p) for p in plans)
    nch += (-nch) % G_CH
    ngroups = nch // G_CH

    sorted_t = []
    for t in range(ntyp):
        o = np.argsort(dst_t[t], kind="stable")
        sorted_t.append((dst_t[t][o], [srcslot_t[t][s][o] for s in range(t + 1)]))
    streams = [_build_streams(plans[c], nch, dst_t, srcslot_t, counts_t, bank_sizes,
                              sorted_t) for c in range(NCORES)]

    per_chunk_bank = [sum(SLOT_CAPS[si][b] for si in range(NSLOT)) * P for b in range(4)]
    bank_base = [[sum(SLOT_CAPS[sj][b] for sj in range(si)) * P for si in range(NSLOT)]
                 for b in range(4)]
    bank_region = [G_CH * per_chunk_bank[b] for b in range(4)]
    bank_tiles = [r // P for r in bank_region]

    x_pad = np.vstack([x, np.zeros((P, D), np.float32)])
    # per-row int8 quant with bf16 scale, packed 130B/row
    s_row = np.maximum(np.abs(x_pad).max(axis=1, keepdims=True), 1e-20) / 126.5
    s_bf = s_row.astype(ml_dtypes.bfloat16)
    x_q = np.round(x_pad / s_row).astype(np.int8)
    x_pk = np.zeros((n_pad, XROW), np.uint8)
    x_pk[:, 0:D] = x_q.view(np.uint8)
    x_pk[:, D:D + 2] = s_bf.view(np.uint8)
    iota = np.tile(np.arange(P, dtype=np.float32), (P, 1))
    ones_row = np.ones((1, P), np.float32)
    wt_f32 = np.ascontiguousarray(W_slots.transpose(1, 0, 2)).astype(np.float32)

    # ---- single u8 blob param for all per-core streams (fewer PJRT params =
    # fewer per-array axon transfers); 256B-aligned sections ----
    def _al(o):
        return (o + 255) & ~255
    sec = {}
    off = 0
    for b in range(nb_banks):
        nby = ngroups * 16 * (bank_region[b] // 16) * 2
        sec[f"idx{b}"] = (off, nby)
        off = _al(off + nby)
    nby = ngroups * P * G_CH * TILES_CHUNK
    sec["dst"] = (off, nby)
    off = _al(off + nby)
    nby = ngroups * P * G_CH * ntyp * 4
    sec["r"] = (off, nby)
    blob_bytes = _al(off + nby)

    # ---- build program ----
    nc = bacc.Bacc("TRN2", target_bir_lowering=False, debug=False,
                   num_devices=NCORES)
    dt = mybir.dt
    xs_d = nc.declare_dram_parameter("xs", [shard_b], dt.uint8, isOutput=False)
    blob_d = nc.declare_dram_parameter("blob", [blob_bytes], dt.uint8, isOutput=False)
    # output: int8 rows + per-row f32 scales packed in one u8 tensor
    out_qb = nch * P * D
    out_bytes = out_qb + nch * P * 4
    out_d = nc.declare_dram_parameter("out", [out_bytes], dt.uint8, isOutput=True)
    outq_ap = out_d[0:out_qb].rearrange(
        "(g c p d) -> g p c d", g=ngroups, c=G_CH, p=P)
    outs_ap = out_d[out_qb:out_bytes].bitcast(dt.float32).rearrange(
        "(g c p) -> g p c", g=ngroups, c=G_CH)

    def _sec_ap(name, dtype, pat, **axes):
        o, n = sec[name]
        ap = blob_d[o:o + n]
        if dtype != dt.uint8:
            ap = ap.bitcast(dtype)
        return ap.rearrange(pat, **axes)

    idx_ap = [_sec_ap(f"idx{b}", dt.int16, "(g p k) -> g p k", g=ngroups, p=16)
              for b in range(nb_banks)]
    dst_ap = _sec_ap("dst", dt.uint8, "(g p c k) -> g p c k",
                     g=ngroups, p=P, c=G_CH)
    r_ap = _sec_ap("r", dt.float32, "(g p c k) -> g p c k",
                   g=ngroups, p=P, c=G_CH)

    AFx = None  # (constants now ride the x AllGather; iota is generated on-device)

    AF = mybir.ActivationFunctionType
    AL = mybir.AluOpType

    with tile.TileContext(nc) as tc:
        with (
            tc.tile_pool(name="const", bufs=1) as cpool,
            tc.tile_pool(name="dram", bufs=1, space="DRAM") as dram,
            tc.tile_pool(name="sbuf", bufs=2) as sb,
            tc.tile_pool(name="psum", bufs=2, space="PSUM") as ps,
        ):
            # x: shard -> bounce -> AllGather into full byte scratch
            x_in = dram.tile([shard_b], dt.uint8)
            x_full = dram.tile([x_bytes], dt.uint8)
            nc.gpsimd.dma_start(out=x_in[:], in_=xs_d[:])
            nc.gpsimd.collective_compute(
                "AllGather", AL.bypass,
                replica_groups=[list(range(NCORES))],
                ins=[x_in[:].opt()], outs=[x_full[:].opt()])
            # expand packed 130B rows to 256B-strided rows (dma_gather needs
            # a 256B-multiple row stride); split DMAs to fit 16-bit AP fields
            x_exp = dram.tile([n_pad, XG], dt.uint8)
            x_pk_rows = x_full[0:n_pad * XROW].rearrange("(n b) -> n b", b=XROW)
            for r0 in range(0, n_pad, BANK):
                r1 = min(r0 + BANK, n_pad)
                nc.sync.dma_start(out=x_exp[r0:r1, 0:XROW],
                                  in_=x_pk_rows[r0:r1])
            x_rows = x_exp[:]

            w_t = cpool.tile([P, NSLOT, D], dt.float32)
            nc.sync.dma_start(
                out=w_t[:],
                in_=x_full[w_off:w_off + NSLOT * P * D * 4].bitcast(dt.float32)
                .rearrange("(p w d) -> p w d", w=NSLOT, d=D))
            io_t = cpool.tile([P, P], dt.float32)
            nc.sync.dma_start(
                out=io_t[:],
                in_=x_full[io_off:io_off + P * P * 4].bitcast(dt.float32)
                .rearrange("(p d) -> p d", d=P))
            on_t = cpool.tile([1, P], dt.float32)
            nc.sync.dma_start(
                out=on_t[:],
                in_=x_full[on_off:on_off + D * 4].bitcast(dt.float32)
                .rearrange("(a d) -> a d", a=1))
            bc_t = cpool.tile([1, P], dt.float32)
            nc.sync.dma_start(
                out=bc_t[:],
                in_=x_full[bc_off:bc_off + D * 4].bitcast(dt.float32)
                .rearrange("(a d) -> a d", a=1))

            with tc.For_i(0, ngroups) as gv:
                graw = []
                gtiles = []
                for b in range(nb_banks):
                    gu = sb.tile([P, bank_tiles[b], XG], dt.uint8, tag=f"gu{b}")
                    gt = sb.tile([P, bank_tiles[b], D], dt.bfloat16, tag=f"g{b}")
                    it = sb.tile([P, bank_region[b] // 16], dt.int16, tag=f"i{b}")
                    nc.sync.dma_start(out=it[0:16, :], in_=idx_ap[b][gv])
                    nc.sync.dma_start(out=it[16:32, :], in_=it[0:16, :])
                    nc.sync.dma_start(out=it[32:64, :], in_=it[0:32, :])
                    nc.sync.dma_start(out=it[64:128, :], in_=it[0:64, :])
                    if STAGE < 1 or bank_sizes[b] == 0:
                        nc.gpsimd.memset(gt[:], 0.0)
                        nc.gpsimd.memset(gu[:], 0.0)
                        graw.append(gu)
                        gtiles.append(gt)
                        continue
                    GMAX = 1024
                    for off in range(0, bank_region[b], GMAX):
                        n = min(GMAX, bank_region[b] - off)
                        nc.gpsimd.dma_gather(
                            out_ap=gu[:, off // P:(off + n) // P, :],
                            in_ap=x_rows[b * BANK: b * BANK + bank_sizes[b], :],
                            idxs_ap=it[:, off // 16:(off + n) // 16],
                            num_idxs=n, num_idxs_reg=n, elem_size=XG)
                    # int8 payload -> bf16 G (exact); row scales fold into R
                    nc.vector.tensor_copy(out=gt[:], in_=gu[:, :, 0:D].bitcast(dt.int8))
                    graw.append(gu)
                    gtiles.append(gt)
                dst_u8 = sb.tile([P, G_CH, TILES_CHUNK], dt.uint8, tag="dst8")
                nc.sync.dma_start(out=dst_u8[:], in_=dst_ap[gv])
                dst_tl = sb.tile([P, G_CH, TILES_CHUNK], dt.float32, tag="dst")
                nc.vector.tensor_copy(out=dst_tl[:], in_=dst_u8[:])
                r_tl = sb.tile([P, G_CH, ntyp], dt.float32, tag="r")
                nc.sync.dma_start(out=r_tl[:], in_=r_ap[gv])
                out_tl = sb.tile([P, G_CH, D], dt.float32, tag="out")
                q8_tl = sb.tile([P, G_CH, D], dt.uint8, tag="q8")
                rmax_tl = sb.tile([P, G_CH, 1], dt.float32, tag="rmax")
                s_tl = sb.tile([P, G_CH], dt.float32, tag="sc")
                m_tl = sb.tile([P, G_CH], dt.float32, tag="mt")

                for cig in range(G_CH):
                    if STAGE < 2:
                        nc.vector.tensor_copy(out=out_tl[:, cig, :], in_=io_t[:])
                        continue
                    # R build: one DVE op per slot over its tiles, then fold the
                    # per-gather-row int8 scales into R (segment-wise)
                    rt_tiles = {}
                    for si in range(NSLOT):
                        nt = SLOT_TILES[si]
                        rt = sb.tile([P, nt, P], dt.bfloat16, tag=f"R{si}")
                        nc.vector.tensor_tensor(
                            out=rt[:],
                            in0=dst_tl[:, cig, TILE_OFF[si]:TILE_OFF[si] + nt, None]
                                .to_broadcast([P, nt, P]),
                            in1=io_t[:, None, :].to_broadcast([P, nt, P]),
                            op=AL.is_equal)
                        k = 0
                        for b in range(nb_banks):
                            nt_b = SLOT_CAPS[si][b]
                            if nt_b == 0:
                                continue
                            base_t = (bank_base[b][si] + cig * per_chunk_bank[b]) // P
                            sc = graw[b][:, base_t:base_t + nt_b, D:D + 2] \
                                .bitcast(dt.bfloat16)
                            nc.vector.tensor_tensor(
                                out=rt[:, k:k + nt_b, :],
                                in0=rt[:, k:k + nt_b, :],
                                in1=sc.to_broadcast([P, nt_b, P]),
                                op=AL.mult)
                            k += nt_b
                        rt_tiles[si] = rt
                    if STAGE < 3:
                        pass
                    # H accumulation
                    h_ps_a = ps.tile([P, 4 * P], dt.float32, space="PSUM", tag="ha")
                    h_ps_b = ps.tile([P, 3 * P], dt.float32, space="PSUM", tag="hb")
                    hmap = {}
                    for si in range(NSLOT):
                        if si < 4:
                            hmap[si] = h_ps_a[:, si * P:(si + 1) * P]
                        else:
                            hmap[si] = h_ps_b[:, (si - 4) * P:(si - 3) * P]
                    if STAGE < 3:
                        nc.vector.tensor_copy(out=out_tl[:, cig, :], in_=rt_tiles[0][:, 0, :])
                        continue
                    # one accumulation group per PSUM bank (start zeroes 2KB bank)
                    mm_a = []  # (out_slice, lhsT, rhs) for bank a (slots 0-3)
                    mm_b = []  # bank b (slots 4,5,6)
                    for si in range(NSLOT):
                        k = 0
                        for b in range(nb_banks):
                            base_t = (bank_base[b][si] + cig * per_chunk_bank[b]) // P
                            for tb in range(SLOT_CAPS[si][b]):
                                trip = (hmap[si], gtiles[b][:, base_t + tb, :],
                                        rt_tiles[si][:, k, :])
                                (mm_a if si < 4 else mm_b).append(trip)
                                k += 1
                    for mms in (mm_a, mm_b):
                        for i, (o, l, rr_) in enumerate(mms):
                            nc.tensor.matmul(out=o, lhsT=l, rhs=rr_,
                                             start=(i == 0), stop=(i == len(mms) - 1))
                    if STAGE < 4:
                        nc.scalar.activation(out=out_tl[:, cig, :], in_=h_ps_a[:, 0:P], func=AF.Copy)
                        continue
                    h_sb_a = sb.tile([P, 4 * P], dt.float32, tag="hsa")
                    nc.scalar.activation(out=h_sb_a[:], in_=h_ps_a[:], func=AF.Copy)
                    h_sb_b = sb.tile([P, 3 * P], dt.float32, tag="hsb")
                    nc.scalar.activation(out=h_sb_b[:], in_=h_ps_b[:], func=AF.Copy)
                    hs = {}
                    for si in range(NSLOT):
                        if si < 4:
                            hs[si] = h_sb_a[:, si * P:(si + 1) * P]
                        else:
                            hs[si] = h_sb_b[:, (si - 4) * P:(si - 3) * P]
                    # agg psum: [t0, t1, t2, self]
                    agg = ps.tile([P, 4 * P], dt.float32, space="PSUM", tag="agg")
                    mm_g = [(agg[:, 3 * P:4 * P], on_t[:], bc_t[:]),
                            (agg[:, 3 * P:4 * P], hs[NSLOT - 1], w_t[:, NSLOT - 1, :])]
                    slot_of_type = {0: [0], 1: [1, 2], 2: [3, 4, 5]}
                    for t in range(ntyp):
                        for si in slot_of_type[t]:
                            mm_g.append((agg[:, t * P:(t + 1) * P], hs[si], w_t[:, si, :]))
                    for i, (o, l, rr_) in enumerate(mm_g):
                        nc.tensor.matmul(out=o, lhsT=l, rhs=rr_,
                                         start=(i == 0), stop=(i == len(mm_g) - 1))
                    # combine: out = self + sum_t r_t * agg_t  (one PSUM input per op)
                    nc.scalar.activation(out=out_tl[:, cig, :], in_=agg[:, 3 * P:4 * P],
                                         func=AF.Copy)
                    for t in range(0, ntyp):
                        nc.vector.scalar_tensor_tensor(
                            out=out_tl[:, cig, :], in0=agg[:, t * P:(t + 1) * P],
                            scalar=r_tl[:, cig, t:t + 1], in1=out_tl[:, cig, :],
                            op0=AL.mult, op1=AL.add)
                # int8 quantization with per-row scale (host dequantizes)
                nc.vector.tensor_reduce(out=rmax_tl[:], in_=out_tl[:],
                                        axis=mybir.AxisListType.X,
                                        op=AL.max, apply_absolute_value=True)
                nc.vector.tensor_scalar(out=s_tl[:], in0=rmax_tl[:, :, 0],
                                        scalar1=1e-20, scalar2=1.0 / 126.5,
                                        op0=AL.max, op1=AL.mult)
                nc.vector.reciprocal(out=m_tl[:], in_=s_tl[:])
                for cig in range(G_CH):
                    # u8 = out/s + 127.5 in [1,254]; trunc==floor==round-half
                    nc.vector.tensor_scalar(
                        out=q8_tl[:, cig, :], in0=out_tl[:, cig, :],
                        scalar1=m_tl[:, cig:cig + 1], scalar2=127.5,
                        op0=AL.mult, op1=AL.add)
                nc.sync.dma_start(out=outq_ap[gv], in_=q8_tl[:])
                nc.sync.dma_start(out=outs_ap[gv], in_=s_tl[:])
    nc.finalize()

    x_aug = np.zeros(x_bytes, np.uint8)
    x_aug[0:n_pad * XROW] = x_pk.ravel()
    x_aug[w_off:w_off + wt_f32.nbytes] = np.frombuffer(wt_f32.tobytes(), np.uint8)
    x_aug[bc_off:bc_off + D * 4] = np.frombuffer(
        bC.reshape(1, D).astype(np.float32).tobytes(), np.uint8)
    x_aug[on_off:on_off + D * 4] = np.frombuffer(ones_row.tobytes(), np.uint8)
    x_aug[io_off:io_off + P * P * 4] = np.frombuffer(iota.tobytes(), np.uint8)

    in_maps = []
    for c in range(NCORES):
        idx_streams, dst_stream, r_arr, selfbase, meta = streams[c]
        blob = np.zeros(blob_bytes, np.uint8)
        for b in range(nb_banks):
            o, n = sec[f"idx{b}"]
            pk = np.stack([_wrap16(idx_streams[b][g]) for g in range(ngroups)])
            blob[o:o + n] = pk.view(np.uint8).ravel()
        o, n = sec["dst"]
        blob[o:o + n] = (dst_stream.reshape(ngroups, G_CH, TILES_CHUNK, P)
                         .transpose(0, 3, 1, 2).reshape(-1))
        o, n = sec["r"]
        blob[o:o + n] = (np.ascontiguousarray(
            r_arr.reshape(ngroups, G_CH, ntyp, P).transpose(0, 3, 1, 2))
            .view(np.uint8).ravel())
        in_maps.append(dict(xs=x_aug[c * shard_b:(c + 1) * shard_b].copy(),
                            blob=blob))

    global _LAST_NC, _LAST_INMAPS
    _LAST_NC, _LAST_INMAPS = nc, in_maps
    if sim:
        from concourse import bass_interp
        s = bass_interp.MultiCoreSim(nc, NCORES)
        for c in range(NCORES):
            for k, v in in_maps[c].items():
                s.cores[c].tensor(k)[:] = v
        s.simulate()
        results = [{"out": np.asarray(s.cores[c].tensor("out")).copy()}
                   for c in range(NCORES)]
        rr = type("R", (), {})(); rr.results = results; rr.exec_time_ns = None
        rr.sim = s
    else:
        import time as _time
        rr = run_bass_kernel_spmd(nc, in_maps, core_ids=list(range(NCORES)))
        if os.environ.get("KBENCH", "0") == "1":
            import gc
            # collect between calls (untimed): prior calls' jit executables and
            # result buffers otherwise linger and inflate later timed calls
            gc.collect()
            t0 = _time.time()
            rr = run_bass_kernel_spmd(nc, in_maps, core_ids=list(range(NCORES)))
            t1 = _time.time()
            print(f"warm call wall: {(t1-t0)*1e3:.1f} ms")
            rr = None
            gc.collect()
            t0 = _time.time()
            rr = run_bass_kernel_spmd(nc, in_maps, core_ids=list(range(NCORES)))
            t1 = _time.time()
            print(f"warm call 2 wall: {(t1-t0)*1e3:.1f} ms")
            print(f"HW exec time: {int((t1-t0)*1e9)} ns")

    out_qb = nch * P * D
    out_full = np.zeros((n_nodes, D), np.float32)
    for c in range(NCORES):
        _, _, _, _, meta = streams[c]
        buf = np.asarray(rr.results[c]["out"]).view(np.uint8).ravel()
        q = buf[:out_qb].reshape(nch, P, D).astype(np.float32) - 127.0
        s = buf[out_qb:out_qb + nch * P * 4].view(np.float32).reshape(nch, P)
        o = q * s[:, :, None]
        for ci, (lo, hi) in enumerate(meta):
            if hi > lo:
                out_full[lo:hi] = o[ci, :hi - lo]
    return out_full, rr


def kernel(x, src0, dst0, src1, dst1, src2, dst2, WA0, WA1, WA2, WC, bC):
    x = np.asarray(x, np.float32)
    n_nodes = x.shape[0]
    dst_t = [np.asarray(d, np.int64) for d in (dst0, dst1, dst2)]
    srcs = [np.asarray(s, np.int64) for s in (src0, src1, src2)]
    srcslot_t = [[srcs[t].reshape(-1, t + 1)[:, s] for s in range(t + 1)]
                 for t in range(3)]
    W_slots = np.stack([
        np.asarray(WA0, np.float32)[0:P],
        np.asarray(WA1, np.float32)[0:P], np.asarray(WA1, np.float32)[P:2 * P],
        np.asarray(WA2, np.float32)[0:P], np.asarray(WA2, np.float32)[P:2 * P],
        np.asarray(WA2, np.float32)[2 * P:3 * P],
        np.asarray(WC, np.float32).T.copy(),
    ])
    out, _ = _run(x, dst_t, srcslot_t, W_slots, None, np.asarray(bC, np.float32),
                  n_nodes)
    return out
